# revision 40
# baseline (speedup 1.0000x reference)
"""MLA attention distributed over 8 TRN2 NeuronCores.

Sharding: tensor-parallel over heads (4 head-groups) x data-parallel over
batch (2). Each core computes, for its (batch, head-group):
  - the shared low-rank compressions c_kv/c_q and the rope key (replicated
    within a batch group),
  - K/V/Q up-projections for its 4 heads,
  - full attention for its 4 heads over all 2048 query positions,
  - a partial output projection (its heads' rows of W_O).
Host gather sums the 4 partial outputs per batch (row-parallel unshard).

Key structure (v3):
  - The rope score q_r.k_r is head-independent: computed once per
    (query-chunk, key-block-pair) and exponentiated once into T_r; per head
    the PE does a single 128-contraction matmul (q_c.k_c), the scalar engine
    exponentiates, and the DVE multiplies by T_r (exp(a+b)=exp(a)exp(b)).
  - The attention mask is folded into V: rows of the augmented [V|1] matrix
    are zeroed for masked keys, which removes the bias operand from every
    exp and makes the softmax denominator (from the ones column) exact.
  - Exps run on [128,1024] psum tiles spanning two banks (two score matmul
    groups per tile) to halve scalar-engine instruction overheads.
  - Query-chunk loop is outermost; the W_O projection of a finished chunk
    overlaps the next chunk's exp work.  Rope is computed per-chunk,
    interleaved into phase 1 / phase 3q so kr/qr are ready early.
"""

from contextlib import ExitStack

import numpy as np

import concourse.bacc as bacc
import concourse.mybir as mybir
import concourse.tile as tile
from concourse.bass_utils import run_bass_kernel_spmd
from concourse.masks import make_identity

B, L, D, H, DC, DH = 2, 2048, 2048, 16, 512, 128
HG = 4                 # head groups (tensor-parallel degree per batch)
HL = H // HG           # heads per core
HDL = HL * DH          # 512 head-dims per core
P = 128
N1 = 512               # matmul free-dim chunk
N2 = 1024              # fat (two-bank) psum tile width
F32 = mybir.dt.float32
BF16 = mybir.dt.bfloat16
F32R = mybir.dt.float32r
SCALE = 1.0 / float(np.sqrt(2 * DH))
M1 = 2 * DC + DH       # 1152: [W_DKV | W_DQ | W_KR] fused output rows
MT1 = M1 // P          # 9
KT1 = D // P           # 16
NCH = L // N1          # 4
KT3 = DC // P          # 4
KB = L // P            # 16 key blocks
KB2 = KB // 2          # 8 key-block pairs


def build_nc(debug=False):
    nc = bacc.Bacc(None, target_bir_lowering=False)

    xT = nc.dram_tensor("xT", [D, L], BF16, kind="ExternalInput")
    w1 = nc.dram_tensor("w1", [D, M1], BF16, kind="ExternalInput")
    wuk = nc.dram_tensor("wuk", [DC, HDL], F32R, kind="ExternalInput")
    w3q = nc.dram_tensor("w3q", [DC, HDL + DH], F32R, kind="ExternalInput")
    wuv = nc.dram_tensor("wuv", [DC, HDL], F32R, kind="ExternalInput")
    wo = nc.dram_tensor("wo", [HDL, D], BF16, kind="ExternalInput")
    # cos cols 0:L / sin cols L:2L on partitions 0:64 (transposed tables)
    cs_d = nc.dram_tensor("csT", [DH // 2, 2 * L], BF16, kind="ExternalInput")
    mask_d = nc.dram_tensor("maskb", [P, KB], F32, kind="ExternalInput")
    out_d = nc.dram_tensor("out", [L, D], F32, kind="ExternalOutput")

    with tile.TileContext(nc) as tc, ExitStack() as es:
        # ---------- constant + psum pools (live whole kernel) ----------
        p_const = es.enter_context(tc.tile_pool(name="const", bufs=1))
        p_ps_g = es.enter_context(tc.tile_pool(name="psg", bufs=2, space="PSUM"))
        p_ps_sc = es.enter_context(tc.tile_pool(name="pssc", bufs=2, space="PSUM"))
        p_ps_av = es.enter_context(tc.tile_pool(name="psav", bufs=2, space="PSUM"))

        km_t = p_const.tile([P, KB], F32, name="km_t")   # 0/1 keep-mask
        nc.sync.dma_start(km_t[:], mask_d[:])
        ident = p_const.tile([P, P], BF16, name="ident")
        make_identity(nc, ident[:])
        warm = p_const.tile([P, 1], F32, name="warm")
        nc.scalar.activation(warm[:], km_t[:, 0:1],
                             mybir.ActivationFunctionType.Exp)

        # ---------- long-lived rope outputs (right-side stack) ----------
        es_krqr = ExitStack()
        p_krqr = es_krqr.enter_context(tc.tile_pool(name="krqrp", bufs=1, side="right"))
        kr_t = [p_krqr.tile([P, N1], BF16, name=f"krT{c}", tag=f"krT{c}")
                for c in range(NCH)]
        qr_t = [p_krqr.tile([P, N1], BF16, name=f"qrT{c}", tag=f"qrT{c}")
                for c in range(NCH)]

        # ---------- phase-1 residents ----------
        es_ckv = ExitStack()
        p_ckv = es_ckv.enter_context(tc.tile_pool(name="ckvp", bufs=1))
        es_cq = ExitStack()
        p_cq = es_cq.enter_context(tc.tile_pool(name="cqp", bufs=1))
        ckv_t = [p_ckv.tile([P, L], F32R, name=f"ckv{i}", tag=f"ckv{i}")
                 for i in range(KT3)]
        cq_t = [p_cq.tile([P, L], F32R, name=f"cq{i}", tag=f"cq{i}")
                for i in range(KT3)]

        # rope tables + pre-rope tiles: dead after the rope work, popped
        # before 3v.
        es_tabxr = ExitStack()
        p_tab = es_tabxr.enter_context(tc.tile_pool(name="tabp", bufs=1))
        p_rope = es_tabxr.enter_context(tc.tile_pool(name="ropep", bufs=1))
        p_xr = es_tabxr.enter_context(tc.tile_pool(name="xrp", bufs=1))
        cs_t = p_tab.tile([DH // 2, 2 * L], BF16, name="cs_t")  # [cos|sin]
        nc.sync.dma_start(cs_t[:], cs_d[:])
        xrk_t = p_xr.tile([P, L], BF16, name="xrkT")

        # one rope chunk: dst[0:64] = xe*c - xo*s ; dst[64:128] = xe*s + xo*c
        # (xo staged down to partitions 0:64, second half staged back up)
        def rope_chunk(src_t, dst_t, ch, pfx, eng):
            cs = slice(ch * N1, (ch + 1) * N1)
            dst = dst_t[ch]
            xe = src_t[0:64, cs]
            xo = p_rope.tile([64, N1], BF16, tag="rxo", name=f"{pfx}xo{ch}")
            nc.gpsimd.dma_start(xo[:], src_t[64:128, cs])
            cc = cs_t[:, ch * N1:(ch + 1) * N1]
            ss = cs_t[:, L + ch * N1:L + (ch + 1) * N1]
            t1 = p_rope.tile([64, N1], F32, tag="rt1", name=f"{pfx}t1{ch}")
            t2 = p_rope.tile([64, N1], F32, tag="rt2", name=f"{pfx}t2{ch}")
            eng.tensor_tensor(t1[:], xe, cc, mybir.AluOpType.mult)
            eng.tensor_tensor(t2[:], xo[:], ss, mybir.AluOpType.mult)
            eng.tensor_tensor(dst[0:64, :], t1[:], t2[:],
                              mybir.AluOpType.subtract)
            t3 = p_rope.tile([64, N1], F32, tag="rt1", name=f"{pfx}t3{ch}")
            t4 = p_rope.tile([64, N1], F32, tag="rt2", name=f"{pfx}t4{ch}")
            eng.tensor_tensor(t3[:], xe, ss, mybir.AluOpType.mult)
            eng.tensor_tensor(t4[:], xo[:], cc, mybir.AluOpType.mult)
            h2 = p_rope.tile([64, N1], BF16, tag="rh2", name=f"{pfx}h2{ch}")
            eng.tensor_tensor(h2[:], t3[:], t4[:], mybir.AluOpType.add)
            nc.gpsimd.dma_start(dst[64:128, :], h2[:])

        # ---------- phase 1: c_kvT | c_qT | xrkT = [Wdkv|Wdq|Wkr].T @ x.T ----
        es_w1 = ExitStack()
        p_w1 = es_w1.enter_context(tc.tile_pool(name="w1p", bufs=1))
        es_xn = ExitStack()
        p_xn = es_xn.enter_context(tc.tile_pool(name="xnp", bufs=34))

        # x tiles alternate sync/vector queues; w1 first-column slices land
        # first (on scalar), then the rest in two stages, so the PE mt-loop
        # is never starved.
        w1_t = []
        xts0 = []
        for kt in range(KT1):
            t = p_xn.tile([P, N1], BF16, tag="xn", name=f"xn_0_{kt}")
            nc.sync.dma_start(t[:], xT[kt * P:(kt + 1) * P, 0:N1])
            xts0.append(t)
            t = p_w1.tile([P, M1], BF16, name=f"w1_{kt}", tag=f"w1_{kt}")
            nc.scalar.dma_start(t[:, 0:P], w1[kt * P:(kt + 1) * P, 0:P])
            w1_t.append(t)
        for lo, hi in ((P, 5 * P), (5 * P, M1)):
            for kt in range(KT1):
                nc.scalar.dma_start(
                    w1_t[kt][:, lo:hi], w1[kt * P:(kt + 1) * P, lo:hi])

        dest1 = ckv_t + cq_t + [xrk_t]
        for nci in range(NCH):
            if nci == 0:
                xts = xts0
            else:
                xts = []
                for kt in range(KT1):
                    t = p_xn.tile([P, N1], BF16, tag="xn", name=f"xn_{nci}_{kt}")
                    nc.sync.dma_start(
                        t[:], xT[kt * P:(kt + 1) * P, nci * N1:(nci + 1) * N1])
                    xts.append(t)
            mt_order = ([4, 5, 6, 7, 0, 1, 2, 3, 8] if nci == NCH - 1
                        else list(range(MT1)))
            for mt in mt_order:
                ps = p_ps_g.tile([P, N1], F32, tag="g", name=f"ps1_{nci}_{mt}")
                for kt in range(KT1):
                    nc.tensor.matmul(ps[:], w1_t[kt][:, mt * P:(mt + 1) * P],
                                     xts[kt][:],
                                     start=(kt == 0), stop=(kt == KT1 - 1))
                # split evictions between DVE and ACT (both idle-ish here)
                if mt % 2 == 0:
                    nc.vector.tensor_copy(dest1[mt][:, nci * N1:(nci + 1) * N1],
                                          ps[:])
                else:
                    nc.scalar.activation(dest1[mt][:, nci * N1:(nci + 1) * N1],
                                         ps[:],
                                         mybir.ActivationFunctionType.Copy)
            # rope-k for this chunk (kr ready long before attention)
            rope_chunk(xrk_t, kr_t, nci, "k", nc.vector)
        es_xn.close()
        es_w1.close()

        es_qc = ExitStack()
        p_qc = es_qc.enter_context(tc.tile_pool(name="qcp", bufs=1, side="right"))
        qc_t = [p_qc.tile([P, L], F32R, tag=f"qc{i}", name=f"qc{i}")
                for i in range(HL)]

        # ---------- phase 3q: q_cT | xrqT = [Wuq_hg|Wqr].T @ c_qT ----------
        es_wuk = ExitStack()
        p_wuk = es_wuk.enter_context(tc.tile_pool(name="wukp", bufs=1))
        wuk_t = []
        for kt in range(KT3):
            t = p_wuk.tile([P, HDL], F32R, tag=f"wuk{kt}", name=f"wuk{kt}")
            nc.scalar.dma_start(t[:], wuk[kt * P:(kt + 1) * P, :])
            wuk_t.append(t)
        es_w3q = ExitStack()
        p_w3q = es_w3q.enter_context(tc.tile_pool(name="w3qp", bufs=1))
        w3q_t = []
        for kt in range(KT3):
            t = p_w3q.tile([P, HDL + DH], F32R, tag=f"w3q{kt}", name=f"w3q{kt}")
            nc.scalar.dma_start(t[:, 0:P], w3q[kt * P:(kt + 1) * P, 0:P])
            w3q_t.append(t)
        for kt in range(KT3):
            nc.scalar.dma_start(w3q_t[kt][:, P:HDL + DH],
                                w3q[kt * P:(kt + 1) * P, P:HDL + DH])
        xrq_t = p_xr.tile([P, L], BF16, name="xrqT")
        dest3 = qc_t + [xrq_t]
        for nci in range(NCH):
            for mt in range(HL + 1):
                pool, tg = ((p_ps_g, "g") if (nci * (HL + 1) + mt) % 2 == 0
                            else (p_ps_sc, "sc"))
                ps = pool.tile([P, N1], F32, tag=tg, name=f"ps3_{nci}_{mt}")
                for kt in range(KT3):
                    nc.tensor.matmul(ps[:], w3q_t[kt][:, mt * P:(mt + 1) * P],
                                     cq_t[kt][:, nci * N1:(nci + 1) * N1],
                                     start=(kt == 0), stop=(kt == KT3 - 1))
                if mt % 2 == 0:
                    nc.vector.tensor_copy(dest3[mt][:, nci * N1:(nci + 1) * N1],
                                          ps[:])
                else:
                    nc.scalar.activation(dest3[mt][:, nci * N1:(nci + 1) * N1],
                                         ps[:],
                                         mybir.ActivationFunctionType.Copy)
            # rope-q for this chunk (qr ready before attention needs it)
            rope_chunk(xrq_t, qr_t, nci, "q", nc.gpsimd)
        es_w3q.close()

        # ---------- phase 3k: k_cT = Wuk_hg.T @ c_kvT ----------
        es_kc = ExitStack()
        p_kc = es_kc.enter_context(tc.tile_pool(name="kcp", bufs=1, side="right"))
        kc_t = [p_kc.tile([P, L], F32R, tag=f"kc{i}", name=f"kc{i}")
                for i in range(HL)]
        for nci in range(NCH):
            for mt in range(HL):
                pool, tg = ((p_ps_g, "g") if (nci * HL + mt) % 2 == 0
                            else (p_ps_sc, "sc"))
                ps = pool.tile([P, N1], F32, tag=tg, name=f"ps3k_{nci}_{mt}")
                for kt in range(KT3):
                    nc.tensor.matmul(ps[:], wuk_t[kt][:, mt * P:(mt + 1) * P],
                                     ckv_t[kt][:, nci * N1:(nci + 1) * N1],
                                     start=(kt == 0), stop=(kt == KT3 - 1))
                if mt % 2 == 0:
                    nc.vector.tensor_copy(kc_t[mt][:, nci * N1:(nci + 1) * N1],
                                          ps[:])
                else:
                    nc.scalar.activation(kc_t[mt][:, nci * N1:(nci + 1) * N1],
                                         ps[:],
                                         mybir.ActivationFunctionType.Copy)
        es_wuk.close()
        es_tabxr.close()   # rope tables + xr + rope temps dead from here
        es_cq.close()

        # ---------- phase 3v: v = c_kv @ Wuv_hg (natural), bf16 + ones col ---
        # masked keys' rows (incl. the ones column) are zeroed -> softmax
        # ignores them and the denominator stays exact.
        es_v = ExitStack()
        p_v = es_v.enter_context(tc.tile_pool(name="vp", bufs=1, side="right"))
        es_wuv = ExitStack()
        p_wuv = es_wuv.enter_context(tc.tile_pool(name="wuvp", bufs=1))
        wuv_t = []
        for kt in range(KT3):
            t = p_wuv.tile([P, HDL], F32R, tag=f"wuv{kt}", name=f"wuv{kt}")
            nc.scalar.dma_start(t[:], wuv[kt * P:(kt + 1) * P, :])
            wuv_t.append(t)
        vaug_t = [p_v.tile([P, HL * (DH + 1)], BF16, tag=f"v{i}", name=f"v{i}")
                  for i in range(KB)]
        # first query-chunk's shared rope score, emitted here so its T_r
        # exps on the scalar engine hide under 3v's PE matmuls
        trs0 = []
        for kb2 in range(KB2):
            ps = p_ps_sc.tile([P, N2], F32, tag="sc", name=f"sr_0_{kb2}")
            for j in range(2):
                kb = 2 * kb2 + j
                nc.tensor.matmul(
                    ps[:, j * N1:(j + 1) * N1],
                    kr_t[kb // 4][:, (kb % 4) * P:(kb % 4 + 1) * P],
                    qr_t[0][:], start=True, stop=True)
            tr = p_v.tile([P, N2], BF16, tag=f"tr0_{kb2}", name=f"tr_0_{kb2}")
            nc.scalar.activation(tr[:], ps[:],
                                 mybir.ActivationFunctionType.Exp,
                                 scale=SCALE)
            trs0.append(tr)
        for mt in range(KB):
            pool, tg = (p_ps_g, "g") if mt % 2 == 0 else (p_ps_sc, "sc")
            ps = pool.tile([P, N1], F32, tag=tg, name=f"psv_{mt}")
            for kt in range(KT3):
                nc.tensor.matmul(ps[:], ckv_t[kt][:, mt * P:(mt + 1) * P],
                                 wuv_t[kt][:],
                                 start=(kt == 0), stop=(kt == KT3 - 1))
            va = vaug_t[mt].rearrange("p (h c) -> p h c", c=DH + 1)
            nc.vector.tensor_copy(va[:, :, 0:DH],
                                  ps.rearrange("p (h c) -> p h c", c=DH))
            nc.vector.memset(va[:, :, DH:DH + 1], 1.0)
            nc.vector.tensor_scalar_mul(vaug_t[mt][:], vaug_t[mt][:],
                                        km_t[:, mt:mt + 1])
        es_wuv.close()
        es_ckv.close()

        # ---------- W_O cache (bf16; queue is idle here) ----------
        p_wo = es.enter_context(tc.tile_pool(name="wop", bufs=1))
        wo_t = {}
        for nci in range(NCH):
            for kt in range(HL):
                t = p_wo.tile([P, N1], BF16, tag=f"wo{nci}_{kt}",
                              name=f"wo_{nci}_{kt}")
                nc.sync.dma_start(t[:], wo[kt * P:(kt + 1) * P,
                                            nci * N1:(nci + 1) * N1])
                wo_t[(nci, kt)] = t

        # ---------- phase 4+5: attention, query-chunk outer ----------
        p_ctx = es.enter_context(tc.tile_pool(name="ctxp", bufs=1))
        es_exp = ExitStack()
        p_e = es_exp.enter_context(tc.tile_pool(name="expp", bufs=12))
        es_tr = ExitStack()
        p_tr = es_tr.enter_context(tc.tile_pool(name="trp", bufs=9))
        es_sm = ExitStack()
        p_sm = es_sm.enter_context(tc.tile_pool(name="smallp", bufs=12))
        es_st = ExitStack()
        p_st = es_st.enter_context(tc.tile_pool(name="stagep", bufs=2))

        ctx_t = [p_ctx.tile([P, L], BF16, tag=f"ctxT{h}", name=f"ctxT{h}")
                 for h in range(HL)]

        # Software-pipelined emission: head h's AV matmuls are interleaved
        # between head h+1's score tiles so the PE has work while the scalar
        # engine exponentiates; the last head's AV interleaves with the NEXT
        # chunk's rope-score matmuls, and phase 5 of the finished chunk runs
        # while the scalar engine starts on the next chunk's T_r/exps.
        def emit_sr_tile(qch, kb2):
            ps = p_ps_sc.tile([P, N2], F32, tag="sc", name=f"sr_{qch}_{kb2}")
            for j in range(2):
                kb = 2 * kb2 + j
                nc.tensor.matmul(
                    ps[:, j * N1:(j + 1) * N1],
                    kr_t[kb // 4][:, (kb % 4) * P:(kb % 4 + 1) * P],
                    qr_t[qch][:], start=True, stop=True)
            tr = p_tr.tile([P, N2], BF16, tag="trT", name=f"tr_{qch}_{kb2}")
            nc.scalar.activation(tr[:], ps[:],
                                 mybir.ActivationFunctionType.Exp,
                                 scale=SCALE)
            return tr

        def emit_score_tile(qch, h, kb2, trs):
            qsl = slice(qch * N1, (qch + 1) * N1)
            ps = p_ps_sc.tile([P, N2], F32, tag="sc",
                              name=f"sc_{h}_{qch}_{kb2}")
            for j in range(2):
                kb = 2 * kb2 + j
                nc.tensor.matmul(ps[:, j * N1:(j + 1) * N1],
                                 kc_t[h][:, kb * P:(kb + 1) * P],
                                 qc_t[h][:, qsl], start=True, stop=True)
            et = p_e.tile([P, N2], BF16, tag="expT",
                          name=f"et_{h}_{qch}_{kb2}")
            nc.scalar.activation(et[:], ps[:],
                                 mybir.ActivationFunctionType.Exp,
                                 scale=SCALE)
            nc.vector.tensor_tensor(et[:], et[:], trs[kb2][:],
                                    mybir.AluOpType.mult)
            return et

        def emit_av_part(qch, h, i, exps, pcs):
            # slot i of 8: qc = i//2, key blocks (i%2)*8 .. +8
            qc = i // 2
            q0 = qch * 4 + qc
            if i % 2 == 0:
                pcs[qc] = p_ps_av.tile([P, DH + 1], F32, tag="av",
                                       name=f"av_{h}_{q0}")
            pc = pcs[qc]
            for kb in range((i % 2) * 8, (i % 2) * 8 + 8):
                nc.tensor.matmul(
                    pc[:],
                    exps[kb // 2][:, (kb % 2) * N1 + qc * P:
                                  (kb % 2) * N1 + (qc + 1) * P],
                    vaug_t[kb][:, h * (DH + 1):(h + 1) * (DH + 1)],
                    start=(kb == 0), stop=(kb == KB - 1))
            if i % 2 == 1:
                rc = p_sm.tile([P, 1], F32, tag="recip", name=f"rc_{h}_{q0}")
                nc.vector.reciprocal(rc[:], pc[:, DH:DH + 1])
                cn = p_sm.tile([P, DH], BF16, tag="cn", name=f"cn_{h}_{q0}")
                nc.vector.tensor_scalar_mul(cn[:], pc[:, 0:DH], rc[:])
                pt = p_ps_g.tile([P, P], BF16, tag="g", name=f"tp_{h}_{q0}")
                nc.tensor.transpose(pt[:], cn[:], ident[:])
                nc.vector.tensor_copy(ctx_t[h][:, q0 * P:(q0 + 1) * P],
                                      pt[:])

        ph5_stg = {}

        def emit_ph5_group(qch, g):
            # one of 16 psum groups for chunk qch: g = qc*4 + nci
            qc, nci = g // NCH, g % NCH
            q0 = qch * 4 + qc
            if nci == 0:
                ph5_stg[q0] = p_st.tile([P, D], F32, tag="stage",
                                        name=f"st_{q0}")
            stg = ph5_stg[q0]
            ps = p_ps_g.tile([P, N1], F32, tag="g", name=f"ps5_{q0}_{nci}")
            for kt in range(HL):
                nc.tensor.matmul(ps[:], ctx_t[kt][:, q0 * P:(q0 + 1) * P],
                                 wo_t[(nci, kt)][:],
                                 start=(kt == 0), stop=(kt == HL - 1))
            nc.vector.tensor_copy(stg[:, nci * N1:(nci + 1) * N1], ps[:])
            if nci == NCH - 1:
                nc.scalar.dma_start(out_d[q0 * P:(q0 + 1) * P, :], stg[:])

        trs = trs0
        for qch in range(NCH):
            exps = [emit_score_tile(qch, 0, kb2, trs) for kb2 in range(KB2)]
            for h in range(HL - 1):
                nexps, pcs = [], {}
                for i in range(KB2):
                    nexps.append(emit_score_tile(qch, h + 1, i, trs))
                    emit_av_part(qch, h, i, exps, pcs)
                exps = nexps
            # last head's AV interleaves with the next chunk's rope scores
            ntrs, pcs = [], {}
            for i in range(KB2):
                if qch + 1 < NCH:
                    ntrs.append(emit_sr_tile(qch + 1, i))
                emit_av_part(qch, HL - 1, i, exps, pcs)
            trs = ntrs
            for g in range(4 * NCH):
                emit_ph5_group(qch, g)

        es_st.close()
        es_sm.close()
        es_tr.close()
        es_exp.close()
        es_v.close()
        es_kc.close()
        es_qc.close()
        es_krqr.close()

    nc.compile()
    return nc


_CACHE = {}


def _get_nc():
    if "nc" not in _CACHE:
        _CACHE["nc"] = build_nc()
    return _CACHE["nc"]


def _BF16(a):
    import ml_dtypes
    return np.asarray(a, dtype=ml_dtypes.bfloat16)


def _host_prep(x, attention_mask, W_DKV, W_DQ, W_UK, W_UV, W_UQ, W_KR, W_QR,
               W_O):
    f = np.float32
    x = np.asarray(x, f)
    attention_mask = np.asarray(attention_mask)
    W_DKV, W_DQ = np.asarray(W_DKV, f), np.asarray(W_DQ, f)
    W_UK, W_UV, W_UQ = np.asarray(W_UK, f), np.asarray(W_UV, f), np.asarray(W_UQ, f)
    W_KR, W_QR, W_O = np.asarray(W_KR, f), np.asarray(W_QR, f), np.asarray(W_O, f)

    perm = np.concatenate([np.arange(0, DH, 2), np.arange(1, DH, 2)])
    w1 = np.ascontiguousarray(
        np.concatenate([W_DKV, W_DQ, W_KR[:, perm]], axis=1))
    xTs = [np.ascontiguousarray(x[b].T) for b in range(B)]

    inv = 1.0 / (10000.0 ** (np.arange(0, DH, 2, dtype=f) / DH))
    freqs = np.arange(L, dtype=f)[:, None] * inv[None, :]
    rope = np.concatenate([np.sin(freqs), np.cos(freqs)], axis=-1).astype(f)
    s_tab, c_tab = rope[:, 0::2], rope[:, 1::2]
    csT = np.ascontiguousarray(
        np.concatenate([c_tab.T, s_tab.T], axis=1))    # [64, 2L]

    maskbs = []
    for b in range(B):
        keep = (attention_mask[b] != 0).astype(f)      # 1=keep, 0=masked
        maskbs.append(np.ascontiguousarray(keep.reshape(KB, P).T))

    in_maps = []
    for c in range(8):
        b, hg = c // HG, c % HG
        cols = slice(hg * HDL, (hg + 1) * HDL)
        in_maps.append({
            "xT": _BF16(xTs[b]),
            "w1": _BF16(w1),
            "wuk": np.ascontiguousarray(W_UK[:, cols]),
            "w3q": np.ascontiguousarray(
                np.concatenate([W_UQ[:, cols], W_QR[:, perm]], axis=1)),
            "wuv": np.ascontiguousarray(W_UV[:, cols]),
            "wo": np.ascontiguousarray(
                _BF16(W_O[hg * HDL:(hg + 1) * HDL, :])),
            "csT": _BF16(csT),
            "maskb": maskbs[b],
        })
    return in_maps


def kernel(x, attention_mask, W_DKV, W_DQ, W_UK, W_UV, W_UQ, W_KR, W_QR, W_O,
           **run_kwargs):
    in_maps = _host_prep(x, attention_mask, W_DKV, W_DQ, W_UK, W_UV, W_UQ,
                         W_KR, W_QR, W_O)
    nc = _get_nc()
    res = run_bass_kernel_spmd(nc, in_maps, core_ids=list(range(8)),
                               **run_kwargs)
    out = np.zeros((B, L, D), np.float32)
    for c in range(8):
        out[c // HG] += res.results[c]["out"]
    if run_kwargs:
        _CACHE["last_results"] = res
    return out


# revision 42
# speedup vs baseline: 1.0209x; 1.0209x over previous
"""MLA attention distributed over 8 TRN2 NeuronCores.

Sharding: tensor-parallel over heads (4 head-groups) x data-parallel over
batch (2). Each core computes, for its (batch, head-group):
  - the shared low-rank compressions c_kv/c_q and the rope key (replicated
    within a batch group),
  - K/V/Q up-projections for its 4 heads,
  - full attention for its 4 heads over all 2048 query positions,
  - a partial output projection (its heads' rows of W_O).
Host gather sums the 4 partial outputs per batch (row-parallel unshard).

Key structure (v3):
  - The rope score q_r.k_r is head-independent: computed once per
    (query-chunk, key-block-pair) and exponentiated once into T_r; per head
    the PE does a single 128-contraction matmul (q_c.k_c), the scalar engine
    exponentiates, and the DVE multiplies by T_r (exp(a+b)=exp(a)exp(b)).
  - The attention mask is folded into V: rows of the augmented [V|1] matrix
    are zeroed for masked keys, which removes the bias operand from every
    exp and makes the softmax denominator (from the ones column) exact.
  - Exps run on [128,1024] psum tiles spanning two banks (two score matmul
    groups per tile) to halve scalar-engine instruction overheads.
  - Query-chunk loop is outermost; the W_O projection of a finished chunk
    overlaps the next chunk's exp work.  Rope is computed per-chunk,
    interleaved into phase 1 / phase 3q so kr/qr are ready early.
"""

from contextlib import ExitStack

import numpy as np

import concourse.bacc as bacc
import concourse.mybir as mybir
import concourse.tile as tile
from concourse.bass_utils import run_bass_kernel_spmd
from concourse.masks import make_identity

B, L, D, H, DC, DH = 2, 2048, 2048, 16, 512, 128
HG = 4                 # head groups (tensor-parallel degree per batch)
HL = H // HG           # heads per core
HDL = HL * DH          # 512 head-dims per core
P = 128
N1 = 512               # matmul free-dim chunk
N2 = 1024              # fat (two-bank) psum tile width
F32 = mybir.dt.float32
BF16 = mybir.dt.bfloat16
F32R = mybir.dt.float32r
SCALE = 1.0 / float(np.sqrt(2 * DH))
M1 = 2 * DC + DH       # 1152: [W_DKV | W_DQ | W_KR] fused output rows
MT1 = M1 // P          # 9
KT1 = D // P           # 16
NCH = L // N1          # 4
KT3 = DC // P          # 4
KB = L // P            # 16 key blocks
KB2 = KB // 2          # 8 key-block pairs


def build_nc(debug=False):
    nc = bacc.Bacc(None, target_bir_lowering=False)

    xT = nc.dram_tensor("xT", [D, L], F32R, kind="ExternalInput")
    w1 = nc.dram_tensor("w1", [D, M1], F32R, kind="ExternalInput")
    wuk = nc.dram_tensor("wuk", [DC, HDL], F32R, kind="ExternalInput")
    w3q = nc.dram_tensor("w3q", [DC, HDL + DH], F32R, kind="ExternalInput")
    wuv = nc.dram_tensor("wuv", [DC, HDL], F32R, kind="ExternalInput")
    wo = nc.dram_tensor("wo", [HDL, D], BF16, kind="ExternalInput")
    # cos cols 0:L / sin cols L:2L on partitions 0:64 (transposed tables)
    cs_d = nc.dram_tensor("csT", [DH // 2, 2 * L], BF16, kind="ExternalInput")
    mask_d = nc.dram_tensor("maskb", [P, KB], F32, kind="ExternalInput")
    out_d = nc.dram_tensor("out", [L, D], F32, kind="ExternalOutput")

    with tile.TileContext(nc) as tc, ExitStack() as es:
        # ---------- constant + psum pools (live whole kernel) ----------
        p_const = es.enter_context(tc.tile_pool(name="const", bufs=1))
        p_ps_g = es.enter_context(tc.tile_pool(name="psg", bufs=2, space="PSUM"))
        p_ps_sc = es.enter_context(tc.tile_pool(name="pssc", bufs=2, space="PSUM"))
        p_ps_av = es.enter_context(tc.tile_pool(name="psav", bufs=2, space="PSUM"))

        km_t = p_const.tile([P, KB], F32, name="km_t")   # 0/1 keep-mask
        nc.sync.dma_start(km_t[:], mask_d[:])
        ident = p_const.tile([P, P], BF16, name="ident")
        make_identity(nc, ident[:])
        warm = p_const.tile([P, 1], F32, name="warm")
        nc.scalar.activation(warm[:], km_t[:, 0:1],
                             mybir.ActivationFunctionType.Exp)

        # ---------- long-lived rope outputs (right-side stack) ----------
        es_krqr = ExitStack()
        p_krqr = es_krqr.enter_context(tc.tile_pool(name="krqrp", bufs=1, side="right"))
        kr_t = [p_krqr.tile([P, N1], BF16, name=f"krT{c}", tag=f"krT{c}")
                for c in range(NCH)]
        qr_t = [p_krqr.tile([P, N1], BF16, name=f"qrT{c}", tag=f"qrT{c}")
                for c in range(NCH)]

        # ---------- phase-1 residents ----------
        es_ckv = ExitStack()
        p_ckv = es_ckv.enter_context(tc.tile_pool(name="ckvp", bufs=1))
        es_cq = ExitStack()
        p_cq = es_cq.enter_context(tc.tile_pool(name="cqp", bufs=1))
        ckv_t = [p_ckv.tile([P, L], F32R, name=f"ckv{i}", tag=f"ckv{i}")
                 for i in range(KT3)]
        cq_t = [p_cq.tile([P, L], F32R, name=f"cq{i}", tag=f"cq{i}")
                for i in range(KT3)]

        # rope tables + pre-rope tiles: dead after the rope work, popped
        # before 3v.
        es_tabxr = ExitStack()
        p_tab = es_tabxr.enter_context(tc.tile_pool(name="tabp", bufs=1))
        p_rope = es_tabxr.enter_context(tc.tile_pool(name="ropep", bufs=1))
        p_xr = es_tabxr.enter_context(tc.tile_pool(name="xrp", bufs=1))
        cs_t = p_tab.tile([DH // 2, 2 * L], BF16, name="cs_t")  # [cos|sin]
        nc.sync.dma_start(cs_t[:], cs_d[:])
        xrk_t = p_xr.tile([P, L], BF16, name="xrkT")

        # one rope chunk: dst[0:64] = xe*c - xo*s ; dst[64:128] = xe*s + xo*c
        # (xo staged down to partitions 0:64, second half staged back up)
        def rope_chunk(src_t, dst_t, ch, pfx, eng):
            cs = slice(ch * N1, (ch + 1) * N1)
            dst = dst_t[ch]
            xe = src_t[0:64, cs]
            xo = p_rope.tile([64, N1], BF16, tag="rxo", name=f"{pfx}xo{ch}")
            nc.gpsimd.dma_start(xo[:], src_t[64:128, cs])
            cc = cs_t[:, ch * N1:(ch + 1) * N1]
            ss = cs_t[:, L + ch * N1:L + (ch + 1) * N1]
            t1 = p_rope.tile([64, N1], F32, tag="rt1", name=f"{pfx}t1{ch}")
            t2 = p_rope.tile([64, N1], F32, tag="rt2", name=f"{pfx}t2{ch}")
            eng.tensor_tensor(t1[:], xe, cc, mybir.AluOpType.mult)
            eng.tensor_tensor(t2[:], xo[:], ss, mybir.AluOpType.mult)
            eng.tensor_tensor(dst[0:64, :], t1[:], t2[:],
                              mybir.AluOpType.subtract)
            t3 = p_rope.tile([64, N1], F32, tag="rt1", name=f"{pfx}t3{ch}")
            t4 = p_rope.tile([64, N1], F32, tag="rt2", name=f"{pfx}t4{ch}")
            eng.tensor_tensor(t3[:], xe, ss, mybir.AluOpType.mult)
            eng.tensor_tensor(t4[:], xo[:], cc, mybir.AluOpType.mult)
            h2 = p_rope.tile([64, N1], BF16, tag="rh2", name=f"{pfx}h2{ch}")
            eng.tensor_tensor(h2[:], t3[:], t4[:], mybir.AluOpType.add)
            nc.gpsimd.dma_start(dst[64:128, :], h2[:])

        # ---------- phase 1: c_kvT | c_qT | xrkT = [Wdkv|Wdq|Wkr].T @ x.T ----
        es_w1 = ExitStack()
        p_w1 = es_w1.enter_context(tc.tile_pool(name="w1p", bufs=1))
        es_xn = ExitStack()
        p_xn = es_xn.enter_context(tc.tile_pool(name="xnp", bufs=20))

        # x tiles alternate sync/vector queues; w1 first-column slices land
        # first (on scalar), then the rest in two stages, so the PE mt-loop
        # is never starved.
        w1_t = []
        xts0 = []
        for kt in range(KT1):
            t = p_xn.tile([P, N1], F32R, tag="xn", name=f"xn_0_{kt}")
            nc.sync.dma_start(t[:], xT[kt * P:(kt + 1) * P, 0:N1])
            xts0.append(t)
            t = p_w1.tile([P, M1], F32R, name=f"w1_{kt}", tag=f"w1_{kt}")
            nc.scalar.dma_start(t[:, 0:P], w1[kt * P:(kt + 1) * P, 0:P])
            w1_t.append(t)
        for lo, hi in ((P, 5 * P), (5 * P, M1)):
            for kt in range(KT1):
                nc.scalar.dma_start(
                    w1_t[kt][:, lo:hi], w1[kt * P:(kt + 1) * P, lo:hi])

        dest1 = ckv_t + cq_t + [xrk_t]
        for nci in range(NCH):
            if nci == 0:
                xts = xts0
            else:
                xts = []
                for kt in range(KT1):
                    t = p_xn.tile([P, N1], F32R, tag="xn", name=f"xn_{nci}_{kt}")
                    nc.sync.dma_start(
                        t[:], xT[kt * P:(kt + 1) * P, nci * N1:(nci + 1) * N1])
                    xts.append(t)
            mt_order = ([4, 5, 6, 7, 0, 1, 2, 3, 8] if nci == NCH - 1
                        else list(range(MT1)))
            for mt in mt_order:
                ps = p_ps_g.tile([P, N1], F32, tag="g", name=f"ps1_{nci}_{mt}")
                for kt in range(KT1):
                    nc.tensor.matmul(ps[:], w1_t[kt][:, mt * P:(mt + 1) * P],
                                     xts[kt][:],
                                     start=(kt == 0), stop=(kt == KT1 - 1))
                # split evictions between DVE and ACT (both idle-ish here)
                if mt % 2 == 0:
                    nc.vector.tensor_copy(dest1[mt][:, nci * N1:(nci + 1) * N1],
                                          ps[:])
                else:
                    nc.scalar.activation(dest1[mt][:, nci * N1:(nci + 1) * N1],
                                         ps[:],
                                         mybir.ActivationFunctionType.Copy)
            # rope-k for this chunk (kr ready long before attention)
            rope_chunk(xrk_t, kr_t, nci, "k", nc.vector)
        es_xn.close()
        es_w1.close()

        es_qc = ExitStack()
        p_qc = es_qc.enter_context(tc.tile_pool(name="qcp", bufs=1, side="right"))
        qc_t = [p_qc.tile([P, L], F32R, tag=f"qc{i}", name=f"qc{i}")
                for i in range(HL)]

        # ---------- phase 3q: q_cT | xrqT = [Wuq_hg|Wqr].T @ c_qT ----------
        es_wuk = ExitStack()
        p_wuk = es_wuk.enter_context(tc.tile_pool(name="wukp", bufs=1))
        wuk_t = []
        for kt in range(KT3):
            t = p_wuk.tile([P, HDL], F32R, tag=f"wuk{kt}", name=f"wuk{kt}")
            nc.scalar.dma_start(t[:], wuk[kt * P:(kt + 1) * P, :])
            wuk_t.append(t)
        es_w3q = ExitStack()
        p_w3q = es_w3q.enter_context(tc.tile_pool(name="w3qp", bufs=1))
        w3q_t = []
        for kt in range(KT3):
            t = p_w3q.tile([P, HDL + DH], F32R, tag=f"w3q{kt}", name=f"w3q{kt}")
            nc.scalar.dma_start(t[:, 0:P], w3q[kt * P:(kt + 1) * P, 0:P])
            w3q_t.append(t)
        for kt in range(KT3):
            nc.scalar.dma_start(w3q_t[kt][:, P:HDL + DH],
                                w3q[kt * P:(kt + 1) * P, P:HDL + DH])
        xrq_t = p_xr.tile([P, L], BF16, name="xrqT")
        dest3 = qc_t + [xrq_t]
        for nci in range(NCH):
            for mt in range(HL + 1):
                pool, tg = ((p_ps_g, "g") if (nci * (HL + 1) + mt) % 2 == 0
                            else (p_ps_sc, "sc"))
                ps = pool.tile([P, N1], F32, tag=tg, name=f"ps3_{nci}_{mt}")
                for kt in range(KT3):
                    nc.tensor.matmul(ps[:], w3q_t[kt][:, mt * P:(mt + 1) * P],
                                     cq_t[kt][:, nci * N1:(nci + 1) * N1],
                                     start=(kt == 0), stop=(kt == KT3 - 1))
                if mt % 2 == 0:
                    nc.vector.tensor_copy(dest3[mt][:, nci * N1:(nci + 1) * N1],
                                          ps[:])
                else:
                    nc.scalar.activation(dest3[mt][:, nci * N1:(nci + 1) * N1],
                                         ps[:],
                                         mybir.ActivationFunctionType.Copy)
            # rope-q for this chunk (qr ready before attention needs it)
            rope_chunk(xrq_t, qr_t, nci, "q", nc.gpsimd)
        es_w3q.close()

        # ---------- phase 3k: k_cT = Wuk_hg.T @ c_kvT ----------
        es_kc = ExitStack()
        p_kc = es_kc.enter_context(tc.tile_pool(name="kcp", bufs=1, side="right"))
        kc_t = [p_kc.tile([P, L], F32R, tag=f"kc{i}", name=f"kc{i}")
                for i in range(HL)]
        for nci in range(NCH):
            for mt in range(HL):
                pool, tg = ((p_ps_g, "g") if (nci * HL + mt) % 2 == 0
                            else (p_ps_sc, "sc"))
                ps = pool.tile([P, N1], F32, tag=tg, name=f"ps3k_{nci}_{mt}")
                for kt in range(KT3):
                    nc.tensor.matmul(ps[:], wuk_t[kt][:, mt * P:(mt + 1) * P],
                                     ckv_t[kt][:, nci * N1:(nci + 1) * N1],
                                     start=(kt == 0), stop=(kt == KT3 - 1))
                if mt % 2 == 0:
                    nc.vector.tensor_copy(kc_t[mt][:, nci * N1:(nci + 1) * N1],
                                          ps[:])
                else:
                    nc.scalar.activation(kc_t[mt][:, nci * N1:(nci + 1) * N1],
                                         ps[:],
                                         mybir.ActivationFunctionType.Copy)
        es_wuk.close()
        es_tabxr.close()   # rope tables + xr + rope temps dead from here
        es_cq.close()

        # ---------- phase 3v: v = c_kv @ Wuv_hg (natural), bf16 + ones col ---
        # masked keys' rows (incl. the ones column) are zeroed -> softmax
        # ignores them and the denominator stays exact.
        es_v = ExitStack()
        p_v = es_v.enter_context(tc.tile_pool(name="vp", bufs=1, side="right"))
        es_wuv = ExitStack()
        p_wuv = es_wuv.enter_context(tc.tile_pool(name="wuvp", bufs=1))
        wuv_t = []
        for kt in range(KT3):
            t = p_wuv.tile([P, HDL], F32R, tag=f"wuv{kt}", name=f"wuv{kt}")
            nc.scalar.dma_start(t[:], wuv[kt * P:(kt + 1) * P, :])
            wuv_t.append(t)
        vaug_t = [p_v.tile([P, HL * (DH + 1)], BF16, tag=f"v{i}", name=f"v{i}")
                  for i in range(KB)]
        # first query-chunk's shared rope score, emitted here so its T_r
        # exps on the scalar engine hide under 3v's PE matmuls
        trs0 = []
        for kb2 in range(KB2):
            ps = p_ps_sc.tile([P, N2], F32, tag="sc", name=f"sr_0_{kb2}")
            for j in range(2):
                kb = 2 * kb2 + j
                nc.tensor.matmul(
                    ps[:, j * N1:(j + 1) * N1],
                    kr_t[kb // 4][:, (kb % 4) * P:(kb % 4 + 1) * P],
                    qr_t[0][:], start=True, stop=True)
            tr = p_v.tile([P, N2], BF16, tag=f"tr0_{kb2}", name=f"tr_0_{kb2}")
            nc.scalar.activation(tr[:], ps[:],
                                 mybir.ActivationFunctionType.Exp,
                                 scale=SCALE)
            trs0.append(tr)
        for mt in range(KB):
            pool, tg = (p_ps_g, "g") if mt % 2 == 0 else (p_ps_sc, "sc")
            ps = pool.tile([P, N1], F32, tag=tg, name=f"psv_{mt}")
            for kt in range(KT3):
                nc.tensor.matmul(ps[:], ckv_t[kt][:, mt * P:(mt + 1) * P],
                                 wuv_t[kt][:],
                                 start=(kt == 0), stop=(kt == KT3 - 1))
            va = vaug_t[mt].rearrange("p (h c) -> p h c", c=DH + 1)
            nc.vector.tensor_copy(va[:, :, 0:DH],
                                  ps.rearrange("p (h c) -> p h c", c=DH))
            nc.vector.memset(va[:, :, DH:DH + 1], 1.0)
            nc.vector.tensor_scalar_mul(vaug_t[mt][:], vaug_t[mt][:],
                                        km_t[:, mt:mt + 1])
        es_wuv.close()
        es_ckv.close()

        # ---------- W_O cache (bf16; queue is idle here) ----------
        p_wo = es.enter_context(tc.tile_pool(name="wop", bufs=1))
        wo_t = {}
        for nci in range(NCH):
            for kt in range(HL):
                t = p_wo.tile([P, N1], BF16, tag=f"wo{nci}_{kt}",
                              name=f"wo_{nci}_{kt}")
                nc.sync.dma_start(t[:], wo[kt * P:(kt + 1) * P,
                                            nci * N1:(nci + 1) * N1])
                wo_t[(nci, kt)] = t

        # ---------- phase 4+5: attention, query-chunk outer ----------
        p_ctx = es.enter_context(tc.tile_pool(name="ctxp", bufs=1))
        es_exp = ExitStack()
        p_e = es_exp.enter_context(tc.tile_pool(name="expp", bufs=12))
        es_tr = ExitStack()
        p_tr = es_tr.enter_context(tc.tile_pool(name="trp", bufs=9))
        es_sm = ExitStack()
        p_sm = es_sm.enter_context(tc.tile_pool(name="smallp", bufs=12))
        es_st = ExitStack()
        p_st = es_st.enter_context(tc.tile_pool(name="stagep", bufs=2))

        ctx_t = [p_ctx.tile([P, L], BF16, tag=f"ctxT{h}", name=f"ctxT{h}")
                 for h in range(HL)]

        # Software-pipelined emission: head h's AV matmuls are interleaved
        # between head h+1's score tiles so the PE has work while the scalar
        # engine exponentiates; the last head's AV interleaves with the NEXT
        # chunk's rope-score matmuls, and phase 5 of the finished chunk runs
        # while the scalar engine starts on the next chunk's T_r/exps.
        def emit_sr_tile(qch, kb2):
            ps = p_ps_sc.tile([P, N2], F32, tag="sc", name=f"sr_{qch}_{kb2}")
            for j in range(2):
                kb = 2 * kb2 + j
                nc.tensor.matmul(
                    ps[:, j * N1:(j + 1) * N1],
                    kr_t[kb // 4][:, (kb % 4) * P:(kb % 4 + 1) * P],
                    qr_t[qch][:], start=True, stop=True)
            tr = p_tr.tile([P, N2], BF16, tag="trT", name=f"tr_{qch}_{kb2}")
            nc.scalar.activation(tr[:], ps[:],
                                 mybir.ActivationFunctionType.Exp,
                                 scale=SCALE)
            return tr

        def emit_score_tile(qch, h, kb2, trs):
            qsl = slice(qch * N1, (qch + 1) * N1)
            ps = p_ps_sc.tile([P, N2], F32, tag="sc",
                              name=f"sc_{h}_{qch}_{kb2}")
            for j in range(2):
                kb = 2 * kb2 + j
                nc.tensor.matmul(ps[:, j * N1:(j + 1) * N1],
                                 kc_t[h][:, kb * P:(kb + 1) * P],
                                 qc_t[h][:, qsl], start=True, stop=True)
            et = p_e.tile([P, N2], BF16, tag="expT",
                          name=f"et_{h}_{qch}_{kb2}")
            nc.scalar.activation(et[:], ps[:],
                                 mybir.ActivationFunctionType.Exp,
                                 scale=SCALE)
            nc.vector.tensor_tensor(et[:], et[:], trs[kb2][:],
                                    mybir.AluOpType.mult)
            return et

        def emit_av_part(qch, h, i, exps, pcs):
            # slot i of 8: qc = i//2, key blocks (i%2)*8 .. +8
            qc = i // 2
            q0 = qch * 4 + qc
            if i % 2 == 0:
                pcs[qc] = p_ps_av.tile([P, DH + 1], F32, tag="av",
                                       name=f"av_{h}_{q0}")
            pc = pcs[qc]
            for kb in range((i % 2) * 8, (i % 2) * 8 + 8):
                nc.tensor.matmul(
                    pc[:],
                    exps[kb // 2][:, (kb % 2) * N1 + qc * P:
                                  (kb % 2) * N1 + (qc + 1) * P],
                    vaug_t[kb][:, h * (DH + 1):(h + 1) * (DH + 1)],
                    start=(kb == 0), stop=(kb == KB - 1))
            if i % 2 == 1:
                rc = p_sm.tile([P, 1], F32, tag="recip", name=f"rc_{h}_{q0}")
                nc.vector.reciprocal(rc[:], pc[:, DH:DH + 1])
                cn = p_sm.tile([P, DH], BF16, tag="cn", name=f"cn_{h}_{q0}")
                nc.vector.tensor_scalar_mul(cn[:], pc[:, 0:DH], rc[:])
                pt = p_ps_g.tile([P, P], BF16, tag="g", name=f"tp_{h}_{q0}")
                nc.tensor.transpose(pt[:], cn[:], ident[:])
                nc.vector.tensor_copy(ctx_t[h][:, q0 * P:(q0 + 1) * P],
                                      pt[:])

        ph5_stg = {}

        def emit_ph5_group(qch, g):
            # one of 16 psum groups for chunk qch: g = qc*4 + nci
            qc, nci = g // NCH, g % NCH
            q0 = qch * 4 + qc
            if nci == 0:
                ph5_stg[q0] = p_st.tile([P, D], F32, tag="stage",
                                        name=f"st_{q0}")
            stg = ph5_stg[q0]
            ps = p_ps_g.tile([P, N1], F32, tag="g", name=f"ps5_{q0}_{nci}")
            for kt in range(HL):
                nc.tensor.matmul(ps[:], ctx_t[kt][:, q0 * P:(q0 + 1) * P],
                                 wo_t[(nci, kt)][:],
                                 start=(kt == 0), stop=(kt == HL - 1))
            # evictions alternate DVE/ACT: the scalar engine is idle in the
            # ph5 block window, halving the psum-rotation latency chain
            if g % 2 == 0:
                nc.vector.tensor_copy(stg[:, nci * N1:(nci + 1) * N1], ps[:])
            else:
                nc.scalar.activation(stg[:, nci * N1:(nci + 1) * N1], ps[:],
                                     mybir.ActivationFunctionType.Copy)
            if nci == NCH - 1:
                nc.scalar.dma_start(out_d[q0 * P:(q0 + 1) * P, :], stg[:])

        trs = trs0
        for qch in range(NCH):
            exps = [emit_score_tile(qch, 0, kb2, trs) for kb2 in range(KB2)]
            for h in range(HL - 1):
                nexps, pcs = [], {}
                for i in range(KB2):
                    nexps.append(emit_score_tile(qch, h + 1, i, trs))
                    emit_av_part(qch, h, i, exps, pcs)
                exps = nexps
            # last head's AV interleaves with the next chunk's rope scores
            ntrs, pcs = [], {}
            for i in range(KB2):
                if qch + 1 < NCH:
                    ntrs.append(emit_sr_tile(qch + 1, i))
                emit_av_part(qch, HL - 1, i, exps, pcs)
            trs = ntrs
            for g in range(4 * NCH):
                emit_ph5_group(qch, g)

        es_st.close()
        es_sm.close()
        es_tr.close()
        es_exp.close()
        es_v.close()
        es_kc.close()
        es_qc.close()
        es_krqr.close()

    nc.compile()
    return nc


_CACHE = {}


def _get_nc():
    if "nc" not in _CACHE:
        _CACHE["nc"] = build_nc()
    return _CACHE["nc"]


def _BF16(a):
    import ml_dtypes
    return np.asarray(a, dtype=ml_dtypes.bfloat16)


def _host_prep(x, attention_mask, W_DKV, W_DQ, W_UK, W_UV, W_UQ, W_KR, W_QR,
               W_O):
    f = np.float32
    x = np.asarray(x, f)
    attention_mask = np.asarray(attention_mask)
    W_DKV, W_DQ = np.asarray(W_DKV, f), np.asarray(W_DQ, f)
    W_UK, W_UV, W_UQ = np.asarray(W_UK, f), np.asarray(W_UV, f), np.asarray(W_UQ, f)
    W_KR, W_QR, W_O = np.asarray(W_KR, f), np.asarray(W_QR, f), np.asarray(W_O, f)

    perm = np.concatenate([np.arange(0, DH, 2), np.arange(1, DH, 2)])
    w1 = np.ascontiguousarray(
        np.concatenate([W_DKV, W_DQ, W_KR[:, perm]], axis=1))
    xTs = [np.ascontiguousarray(x[b].T) for b in range(B)]

    inv = 1.0 / (10000.0 ** (np.arange(0, DH, 2, dtype=f) / DH))
    freqs = np.arange(L, dtype=f)[:, None] * inv[None, :]
    rope = np.concatenate([np.sin(freqs), np.cos(freqs)], axis=-1).astype(f)
    s_tab, c_tab = rope[:, 0::2], rope[:, 1::2]
    csT = np.ascontiguousarray(
        np.concatenate([c_tab.T, s_tab.T], axis=1))    # [64, 2L]

    maskbs = []
    for b in range(B):
        keep = (attention_mask[b] != 0).astype(f)      # 1=keep, 0=masked
        maskbs.append(np.ascontiguousarray(keep.reshape(KB, P).T))

    in_maps = []
    for c in range(8):
        b, hg = c // HG, c % HG
        cols = slice(hg * HDL, (hg + 1) * HDL)
        in_maps.append({
            "xT": xTs[b],
            "w1": w1,
            "wuk": np.ascontiguousarray(W_UK[:, cols]),
            "w3q": np.ascontiguousarray(
                np.concatenate([W_UQ[:, cols], W_QR[:, perm]], axis=1)),
            "wuv": np.ascontiguousarray(W_UV[:, cols]),
            "wo": np.ascontiguousarray(
                _BF16(W_O[hg * HDL:(hg + 1) * HDL, :])),
            "csT": _BF16(csT),
            "maskb": maskbs[b],
        })
    return in_maps


def kernel(x, attention_mask, W_DKV, W_DQ, W_UK, W_UV, W_UQ, W_KR, W_QR, W_O,
           **run_kwargs):
    in_maps = _host_prep(x, attention_mask, W_DKV, W_DQ, W_UK, W_UV, W_UQ,
                         W_KR, W_QR, W_O)
    nc = _get_nc()
    res = run_bass_kernel_spmd(nc, in_maps, core_ids=list(range(8)),
                               **run_kwargs)
    out = np.zeros((B, L, D), np.float32)
    for c in range(8):
        out[c // HG] += res.results[c]["out"]
    if run_kwargs:
        _CACHE["last_results"] = res
    return out


# revision 43
# speedup vs baseline: 1.0253x; 1.0043x over previous
"""MLA attention distributed over 8 TRN2 NeuronCores.

Sharding: tensor-parallel over heads (4 head-groups) x data-parallel over
batch (2). Each core computes, for its (batch, head-group):
  - the shared low-rank compressions c_kv/c_q and the rope key (replicated
    within a batch group),
  - K/V/Q up-projections for its 4 heads,
  - full attention for its 4 heads over all 2048 query positions,
  - a partial output projection (its heads' rows of W_O).
Host gather sums the 4 partial outputs per batch (row-parallel unshard).

Key structure (v3):
  - The rope score q_r.k_r is head-independent: computed once per
    (query-chunk, key-block-pair) and exponentiated once into T_r; per head
    the PE does a single 128-contraction matmul (q_c.k_c), the scalar engine
    exponentiates, and the DVE multiplies by T_r (exp(a+b)=exp(a)exp(b)).
  - The attention mask is folded into V: rows of the augmented [V|1] matrix
    are zeroed for masked keys, which removes the bias operand from every
    exp and makes the softmax denominator (from the ones column) exact.
  - Exps run on [128,1024] psum tiles spanning two banks (two score matmul
    groups per tile) to halve scalar-engine instruction overheads.
  - Query-chunk loop is outermost; the W_O projection of a finished chunk
    overlaps the next chunk's exp work.  Rope is computed per-chunk,
    interleaved into phase 1 / phase 3q so kr/qr are ready early.
"""

from contextlib import ExitStack

import numpy as np

import concourse.bacc as bacc
import concourse.mybir as mybir
import concourse.tile as tile
from concourse.bass_utils import run_bass_kernel_spmd
from concourse.masks import make_identity

B, L, D, H, DC, DH = 2, 2048, 2048, 16, 512, 128
HG = 4                 # head groups (tensor-parallel degree per batch)
HL = H // HG           # heads per core
HDL = HL * DH          # 512 head-dims per core
P = 128
N1 = 512               # matmul free-dim chunk
N2 = 1024              # fat (two-bank) psum tile width
F32 = mybir.dt.float32
BF16 = mybir.dt.bfloat16
F32R = mybir.dt.float32r
SCALE = 1.0 / float(np.sqrt(2 * DH))
M1 = 2 * DC + DH       # 1152: [W_DKV | W_DQ | W_KR] fused output rows
MT1 = M1 // P          # 9
KT1 = D // P           # 16
NCH = L // N1          # 4
KT3 = DC // P          # 4
KB = L // P            # 16 key blocks
KB2 = KB // 2          # 8 key-block pairs


def build_nc(debug=False):
    nc = bacc.Bacc(None, target_bir_lowering=False)

    xT = nc.dram_tensor("xT", [D, L], F32R, kind="ExternalInput")
    w1 = nc.dram_tensor("w1", [D, M1], F32R, kind="ExternalInput")
    wuk = nc.dram_tensor("wuk", [DC, HDL], F32R, kind="ExternalInput")
    w3q = nc.dram_tensor("w3q", [DC, HDL + DH], F32R, kind="ExternalInput")
    wuv = nc.dram_tensor("wuv", [DC, HDL], F32R, kind="ExternalInput")
    wo = nc.dram_tensor("wo", [HDL, D], BF16, kind="ExternalInput")
    # cos cols 0:L / sin cols L:2L on partitions 0:64 (transposed tables)
    cs_d = nc.dram_tensor("csT", [DH // 2, 2 * L], BF16, kind="ExternalInput")
    mask_d = nc.dram_tensor("maskb", [P, KB], F32, kind="ExternalInput")
    out_d = nc.dram_tensor("out", [L, D], F32, kind="ExternalOutput")

    with tile.TileContext(nc) as tc, ExitStack() as es:
        # ---------- constant + psum pools (live whole kernel) ----------
        p_const = es.enter_context(tc.tile_pool(name="const", bufs=1))
        p_ps_g = es.enter_context(tc.tile_pool(name="psg", bufs=2, space="PSUM"))
        p_ps_sc = es.enter_context(tc.tile_pool(name="pssc", bufs=2, space="PSUM"))
        p_ps_av = es.enter_context(tc.tile_pool(name="psav", bufs=2, space="PSUM"))

        km_t = p_const.tile([P, KB], F32, name="km_t")   # 0/1 keep-mask
        nc.sync.dma_start(km_t[:], mask_d[:])
        ident = p_const.tile([P, P], BF16, name="ident")
        make_identity(nc, ident[:])
        warm = p_const.tile([P, 1], F32, name="warm")
        nc.scalar.activation(warm[:], km_t[:, 0:1],
                             mybir.ActivationFunctionType.Exp)

        # ---------- long-lived rope outputs (right-side stack) ----------
        es_krqr = ExitStack()
        p_krqr = es_krqr.enter_context(tc.tile_pool(name="krqrp", bufs=1, side="right"))
        kr_t = [p_krqr.tile([P, N1], BF16, name=f"krT{c}", tag=f"krT{c}")
                for c in range(NCH)]
        qr_t = [p_krqr.tile([P, N1], BF16, name=f"qrT{c}", tag=f"qrT{c}")
                for c in range(NCH)]

        # ---------- phase-1 residents ----------
        es_ckv = ExitStack()
        p_ckv = es_ckv.enter_context(tc.tile_pool(name="ckvp", bufs=1))
        es_cq = ExitStack()
        p_cq = es_cq.enter_context(tc.tile_pool(name="cqp", bufs=1))
        ckv_t = [p_ckv.tile([P, L], F32R, name=f"ckv{i}", tag=f"ckv{i}")
                 for i in range(KT3)]
        cq_t = [p_cq.tile([P, L], F32R, name=f"cq{i}", tag=f"cq{i}")
                for i in range(KT3)]

        # rope tables + pre-rope tiles: dead after the rope work, popped
        # before 3v.
        es_tabxr = ExitStack()
        p_tab = es_tabxr.enter_context(tc.tile_pool(name="tabp", bufs=1))
        p_rope = es_tabxr.enter_context(tc.tile_pool(name="ropep", bufs=1))
        p_xr = es_tabxr.enter_context(tc.tile_pool(name="xrp", bufs=1))
        cs_t = p_tab.tile([DH // 2, 2 * L], BF16, name="cs_t")  # [cos|sin]
        nc.sync.dma_start(cs_t[:], cs_d[:])
        xrk_t = p_xr.tile([P, L], BF16, name="xrkT")

        # one rope chunk: dst[0:64] = xe*c - xo*s ; dst[64:128] = xe*s + xo*c
        # (xo staged down to partitions 0:64, second half staged back up)
        def rope_chunk(src_t, dst_t, ch, pfx, eng):
            cs = slice(ch * N1, (ch + 1) * N1)
            dst = dst_t[ch]
            xe = src_t[0:64, cs]
            xo = p_rope.tile([64, N1], BF16, tag="rxo", name=f"{pfx}xo{ch}")
            nc.gpsimd.dma_start(xo[:], src_t[64:128, cs])
            cc = cs_t[:, ch * N1:(ch + 1) * N1]
            ss = cs_t[:, L + ch * N1:L + (ch + 1) * N1]
            t1 = p_rope.tile([64, N1], F32, tag="rt1", name=f"{pfx}t1{ch}")
            t2 = p_rope.tile([64, N1], F32, tag="rt2", name=f"{pfx}t2{ch}")
            eng.tensor_tensor(t1[:], xe, cc, mybir.AluOpType.mult)
            eng.tensor_tensor(t2[:], xo[:], ss, mybir.AluOpType.mult)
            eng.tensor_tensor(dst[0:64, :], t1[:], t2[:],
                              mybir.AluOpType.subtract)
            t3 = p_rope.tile([64, N1], F32, tag="rt1", name=f"{pfx}t3{ch}")
            t4 = p_rope.tile([64, N1], F32, tag="rt2", name=f"{pfx}t4{ch}")
            eng.tensor_tensor(t3[:], xe, ss, mybir.AluOpType.mult)
            eng.tensor_tensor(t4[:], xo[:], cc, mybir.AluOpType.mult)
            h2 = p_rope.tile([64, N1], BF16, tag="rh2", name=f"{pfx}h2{ch}")
            eng.tensor_tensor(h2[:], t3[:], t4[:], mybir.AluOpType.add)
            nc.gpsimd.dma_start(dst[64:128, :], h2[:])

        # ---------- phase 1: c_kvT | c_qT | xrkT = [Wdkv|Wdq|Wkr].T @ x.T ----
        es_w1 = ExitStack()
        p_w1 = es_w1.enter_context(tc.tile_pool(name="w1p", bufs=1))
        es_xn = ExitStack()
        p_xn = es_xn.enter_context(tc.tile_pool(name="xnp", bufs=20))

        # x tiles alternate sync/vector queues; w1 first-column slices land
        # first (on scalar), then the rest in two stages, so the PE mt-loop
        # is never starved.
        w1_t = []
        xts0 = []
        for kt in range(KT1):
            t = p_xn.tile([P, N1], F32R, tag="xn", name=f"xn_0_{kt}")
            nc.sync.dma_start(t[:], xT[kt * P:(kt + 1) * P, 0:N1])
            xts0.append(t)
            t = p_w1.tile([P, M1], F32R, name=f"w1_{kt}", tag=f"w1_{kt}")
            nc.scalar.dma_start(t[:, 0:P], w1[kt * P:(kt + 1) * P, 0:P])
            w1_t.append(t)
        for lo, hi in ((P, 5 * P), (5 * P, M1)):
            for kt in range(KT1):
                nc.scalar.dma_start(
                    w1_t[kt][:, lo:hi], w1[kt * P:(kt + 1) * P, lo:hi])

        dest1 = ckv_t + cq_t + [xrk_t]
        for nci in range(NCH):
            if nci == 0:
                xts = xts0
            else:
                xts = []
                for kt in range(KT1):
                    t = p_xn.tile([P, N1], F32R, tag="xn", name=f"xn_{nci}_{kt}")
                    nc.sync.dma_start(
                        t[:], xT[kt * P:(kt + 1) * P, nci * N1:(nci + 1) * N1])
                    xts.append(t)
            mt_order = ([4, 5, 6, 7, 0, 1, 2, 3, 8] if nci == NCH - 1
                        else list(range(MT1)))
            for mt in mt_order:
                ps = p_ps_g.tile([P, N1], F32, tag="g", name=f"ps1_{nci}_{mt}")
                for kt in range(KT1):
                    nc.tensor.matmul(ps[:], w1_t[kt][:, mt * P:(mt + 1) * P],
                                     xts[kt][:],
                                     start=(kt == 0), stop=(kt == KT1 - 1))
                # split evictions between DVE and ACT (both idle-ish here)
                if mt % 2 == 0:
                    nc.vector.tensor_copy(dest1[mt][:, nci * N1:(nci + 1) * N1],
                                          ps[:])
                else:
                    nc.scalar.activation(dest1[mt][:, nci * N1:(nci + 1) * N1],
                                         ps[:],
                                         mybir.ActivationFunctionType.Copy)
            # rope-k for this chunk (kr ready long before attention)
            rope_chunk(xrk_t, kr_t, nci, "k", nc.vector)
        es_xn.close()
        es_w1.close()

        es_qc = ExitStack()
        p_qc = es_qc.enter_context(tc.tile_pool(name="qcp", bufs=1, side="right"))
        qc_t = [p_qc.tile([P, L], F32R, tag=f"qc{i}", name=f"qc{i}")
                for i in range(HL)]

        # ---------- phase 3q: q_cT | xrqT = [Wuq_hg|Wqr].T @ c_qT ----------
        es_wuk = ExitStack()
        p_wuk = es_wuk.enter_context(tc.tile_pool(name="wukp", bufs=1))
        wuk_t = []
        for kt in range(KT3):
            t = p_wuk.tile([P, HDL], F32R, tag=f"wuk{kt}", name=f"wuk{kt}")
            nc.scalar.dma_start(t[:], wuk[kt * P:(kt + 1) * P, :])
            wuk_t.append(t)
        es_w3q = ExitStack()
        p_w3q = es_w3q.enter_context(tc.tile_pool(name="w3qp", bufs=1))
        w3q_t = []
        for kt in range(KT3):
            t = p_w3q.tile([P, HDL + DH], F32R, tag=f"w3q{kt}", name=f"w3q{kt}")
            nc.scalar.dma_start(t[:, 0:P], w3q[kt * P:(kt + 1) * P, 0:P])
            w3q_t.append(t)
        for kt in range(KT3):
            nc.scalar.dma_start(w3q_t[kt][:, P:HDL + DH],
                                w3q[kt * P:(kt + 1) * P, P:HDL + DH])
        xrq_t = p_xr.tile([P, L], BF16, name="xrqT")
        dest3 = qc_t + [xrq_t]
        for nci in range(NCH):
            for mt in range(HL + 1):
                pool, tg = ((p_ps_g, "g") if (nci * (HL + 1) + mt) % 2 == 0
                            else (p_ps_sc, "sc"))
                ps = pool.tile([P, N1], F32, tag=tg, name=f"ps3_{nci}_{mt}")
                for kt in range(KT3):
                    nc.tensor.matmul(ps[:], w3q_t[kt][:, mt * P:(mt + 1) * P],
                                     cq_t[kt][:, nci * N1:(nci + 1) * N1],
                                     start=(kt == 0), stop=(kt == KT3 - 1))
                if mt % 2 == 0:
                    nc.vector.tensor_copy(dest3[mt][:, nci * N1:(nci + 1) * N1],
                                          ps[:])
                else:
                    nc.scalar.activation(dest3[mt][:, nci * N1:(nci + 1) * N1],
                                         ps[:],
                                         mybir.ActivationFunctionType.Copy)
            # rope-q for this chunk (qr ready before attention needs it)
            rope_chunk(xrq_t, qr_t, nci, "q", nc.gpsimd)
        es_w3q.close()

        # ---------- phase 3k: k_cT = Wuk_hg.T @ c_kvT ----------
        es_kc = ExitStack()
        p_kc = es_kc.enter_context(tc.tile_pool(name="kcp", bufs=1, side="right"))
        kc_t = [p_kc.tile([P, L], F32R, tag=f"kc{i}", name=f"kc{i}")
                for i in range(HL)]
        for nci in range(NCH):
            for mt in range(HL):
                pool, tg = ((p_ps_g, "g") if (nci * HL + mt) % 2 == 0
                            else (p_ps_sc, "sc"))
                ps = pool.tile([P, N1], F32, tag=tg, name=f"ps3k_{nci}_{mt}")
                for kt in range(KT3):
                    nc.tensor.matmul(ps[:], wuk_t[kt][:, mt * P:(mt + 1) * P],
                                     ckv_t[kt][:, nci * N1:(nci + 1) * N1],
                                     start=(kt == 0), stop=(kt == KT3 - 1))
                if mt % 2 == 0:
                    nc.vector.tensor_copy(kc_t[mt][:, nci * N1:(nci + 1) * N1],
                                          ps[:])
                else:
                    nc.scalar.activation(kc_t[mt][:, nci * N1:(nci + 1) * N1],
                                         ps[:],
                                         mybir.ActivationFunctionType.Copy)
        es_wuk.close()
        es_tabxr.close()   # rope tables + xr + rope temps dead from here
        es_cq.close()

        # ---------- phase 3v: v = c_kv @ Wuv_hg (natural), bf16 + ones col ---
        # masked keys' rows (incl. the ones column) are zeroed -> softmax
        # ignores them and the denominator stays exact.
        es_v = ExitStack()
        p_v = es_v.enter_context(tc.tile_pool(name="vp", bufs=1, side="right"))
        es_wuv = ExitStack()
        p_wuv = es_wuv.enter_context(tc.tile_pool(name="wuvp", bufs=1))
        wuv_t = []
        for kt in range(KT3):
            t = p_wuv.tile([P, HDL], F32R, tag=f"wuv{kt}", name=f"wuv{kt}")
            nc.scalar.dma_start(t[:], wuv[kt * P:(kt + 1) * P, :])
            wuv_t.append(t)
        vaug_t = [p_v.tile([P, HL * (DH + 1)], BF16, tag=f"v{i}", name=f"v{i}")
                  for i in range(KB)]
        # first query-chunk's shared rope score, emitted here so its T_r
        # exps on the scalar engine hide under 3v's PE matmuls
        trs0 = []
        for kb2 in range(KB2):
            ps = p_ps_sc.tile([P, N2], F32, tag="sc", name=f"sr_0_{kb2}")
            for j in range(2):
                kb = 2 * kb2 + j
                nc.tensor.matmul(
                    ps[:, j * N1:(j + 1) * N1],
                    kr_t[kb // 4][:, (kb % 4) * P:(kb % 4 + 1) * P],
                    qr_t[0][:], start=True, stop=True)
            tr = p_v.tile([P, N2], BF16, tag=f"tr0_{kb2}", name=f"tr_0_{kb2}")
            nc.scalar.activation(tr[:], ps[:],
                                 mybir.ActivationFunctionType.Exp,
                                 scale=SCALE)
            trs0.append(tr)
        for mt in range(KB):
            pool, tg = (p_ps_g, "g") if mt % 2 == 0 else (p_ps_sc, "sc")
            ps = pool.tile([P, N1], F32, tag=tg, name=f"psv_{mt}")
            for kt in range(KT3):
                nc.tensor.matmul(ps[:], ckv_t[kt][:, mt * P:(mt + 1) * P],
                                 wuv_t[kt][:],
                                 start=(kt == 0), stop=(kt == KT3 - 1))
            va = vaug_t[mt].rearrange("p (h c) -> p h c", c=DH + 1)
            nc.vector.tensor_copy(va[:, :, 0:DH],
                                  ps.rearrange("p (h c) -> p h c", c=DH))
            nc.vector.memset(va[:, :, DH:DH + 1], 1.0)
            nc.vector.tensor_scalar_mul(vaug_t[mt][:], vaug_t[mt][:],
                                        km_t[:, mt:mt + 1])
        es_wuv.close()
        es_ckv.close()

        # ---------- W_O cache (bf16; queue is idle here) ----------
        p_wo = es.enter_context(tc.tile_pool(name="wop", bufs=1))
        wo_t = {}
        for nci in range(NCH):
            for kt in range(HL):
                t = p_wo.tile([P, N1], BF16, tag=f"wo{nci}_{kt}",
                              name=f"wo_{nci}_{kt}")
                nc.sync.dma_start(t[:], wo[kt * P:(kt + 1) * P,
                                            nci * N1:(nci + 1) * N1])
                wo_t[(nci, kt)] = t

        # ---------- phase 4+5: attention, query-chunk outer ----------
        p_ctx = es.enter_context(tc.tile_pool(name="ctxp", bufs=1))
        es_exp = ExitStack()
        p_e = es_exp.enter_context(tc.tile_pool(name="expp", bufs=12))
        es_tr = ExitStack()
        p_tr = es_tr.enter_context(tc.tile_pool(name="trp", bufs=9))
        es_sm = ExitStack()
        p_sm = es_sm.enter_context(tc.tile_pool(name="smallp", bufs=12))
        es_st = ExitStack()
        p_st = es_st.enter_context(tc.tile_pool(name="stagep", bufs=2))

        ctx_t = [p_ctx.tile([P, L], BF16, tag=f"ctxT{h}", name=f"ctxT{h}")
                 for h in range(HL)]

        # Software-pipelined emission: head h's AV matmuls are interleaved
        # between head h+1's score tiles so the PE has work while the scalar
        # engine exponentiates; the last head's AV interleaves with the NEXT
        # chunk's rope-score matmuls, and phase 5 of the finished chunk runs
        # while the scalar engine starts on the next chunk's T_r/exps.
        def emit_sr_tile(qch, kb2):
            ps = p_ps_sc.tile([P, N2], F32, tag="sc", name=f"sr_{qch}_{kb2}")
            for j in range(2):
                kb = 2 * kb2 + j
                nc.tensor.matmul(
                    ps[:, j * N1:(j + 1) * N1],
                    kr_t[kb // 4][:, (kb % 4) * P:(kb % 4 + 1) * P],
                    qr_t[qch][:], start=True, stop=True)
            tr = p_tr.tile([P, N2], BF16, tag="trT", name=f"tr_{qch}_{kb2}")
            nc.scalar.activation(tr[:], ps[:],
                                 mybir.ActivationFunctionType.Exp,
                                 scale=SCALE)
            return tr

        def emit_score_tile(qch, h, kb2, trs):
            qsl = slice(qch * N1, (qch + 1) * N1)
            ps = p_ps_sc.tile([P, N2], F32, tag="sc",
                              name=f"sc_{h}_{qch}_{kb2}")
            for j in range(2):
                kb = 2 * kb2 + j
                nc.tensor.matmul(ps[:, j * N1:(j + 1) * N1],
                                 kc_t[h][:, kb * P:(kb + 1) * P],
                                 qc_t[h][:, qsl], start=True, stop=True)
            et = p_e.tile([P, N2], BF16, tag="expT",
                          name=f"et_{h}_{qch}_{kb2}")
            nc.scalar.activation(et[:], ps[:],
                                 mybir.ActivationFunctionType.Exp,
                                 scale=SCALE)
            nc.vector.tensor_tensor(et[:], et[:], trs[kb2][:],
                                    mybir.AluOpType.mult)
            return et

        def emit_av_part(qch, h, i, exps, pcs):
            # slot i of 8: qc = i//2, key blocks (i%2)*8 .. +8
            qc = i // 2
            q0 = qch * 4 + qc
            if i % 2 == 0:
                pcs[qc] = p_ps_av.tile([P, DH + 1], F32, tag="av",
                                       name=f"av_{h}_{q0}")
            pc = pcs[qc]
            for kb in range((i % 2) * 8, (i % 2) * 8 + 8):
                nc.tensor.matmul(
                    pc[:],
                    exps[kb // 2][:, (kb % 2) * N1 + qc * P:
                                  (kb % 2) * N1 + (qc + 1) * P],
                    vaug_t[kb][:, h * (DH + 1):(h + 1) * (DH + 1)],
                    start=(kb == 0), stop=(kb == KB - 1))
            if i % 2 == 1:
                rc = p_sm.tile([P, 1], F32, tag="recip", name=f"rc_{h}_{q0}")
                nc.vector.reciprocal(rc[:], pc[:, DH:DH + 1])
                cn = p_sm.tile([P, DH], BF16, tag="cn", name=f"cn_{h}_{q0}")
                nc.vector.tensor_scalar_mul(cn[:], pc[:, 0:DH], rc[:])
                pt = p_ps_g.tile([P, P], BF16, tag="g", name=f"tp_{h}_{q0}")
                nc.tensor.transpose(pt[:], cn[:], ident[:])
                nc.vector.tensor_copy(ctx_t[h][:, q0 * P:(q0 + 1) * P],
                                      pt[:])

        ph5_stg = {}

        def emit_ph5_group(qch, g):
            # one of 16 psum groups for chunk qch: g = qc*4 + nci
            qc, nci = g // NCH, g % NCH
            q0 = qch * 4 + qc
            if nci == 0:
                ph5_stg[q0] = p_st.tile([P, D], F32, tag="stage",
                                        name=f"st_{q0}")
            stg = ph5_stg[q0]
            ps = p_ps_g.tile([P, N1], F32, tag="g", name=f"ps5_{q0}_{nci}")
            for kt in range(HL):
                nc.tensor.matmul(ps[:], ctx_t[kt][:, q0 * P:(q0 + 1) * P],
                                 wo_t[(nci, kt)][:],
                                 start=(kt == 0), stop=(kt == HL - 1))
            nc.vector.tensor_copy(stg[:, nci * N1:(nci + 1) * N1], ps[:])
            if nci == NCH - 1:
                nc.scalar.dma_start(out_d[q0 * P:(q0 + 1) * P, :], stg[:])

        trs = trs0
        for qch in range(NCH):
            exps = [emit_score_tile(qch, 0, kb2, trs) for kb2 in range(KB2)]
            for h in range(HL - 1):
                nexps, pcs = [], {}
                for i in range(KB2):
                    nexps.append(emit_score_tile(qch, h + 1, i, trs))
                    emit_av_part(qch, h, i, exps, pcs)
                exps = nexps
            # last head's AV interleaves with the next chunk's rope scores
            ntrs, pcs = [], {}
            for i in range(KB2):
                if qch + 1 < NCH:
                    ntrs.append(emit_sr_tile(qch + 1, i))
                emit_av_part(qch, HL - 1, i, exps, pcs)
            trs = ntrs
            for g in range(4 * NCH):
                emit_ph5_group(qch, g)

        es_st.close()
        es_sm.close()
        es_tr.close()
        es_exp.close()
        es_v.close()
        es_kc.close()
        es_qc.close()
        es_krqr.close()

    nc.compile()
    return nc


_CACHE = {}


def _get_nc():
    if "nc" not in _CACHE:
        _CACHE["nc"] = build_nc()
    return _CACHE["nc"]


def _BF16(a):
    import ml_dtypes
    return np.asarray(a, dtype=ml_dtypes.bfloat16)


def _host_prep(x, attention_mask, W_DKV, W_DQ, W_UK, W_UV, W_UQ, W_KR, W_QR,
               W_O):
    f = np.float32
    x = np.asarray(x, f)
    attention_mask = np.asarray(attention_mask)
    W_DKV, W_DQ = np.asarray(W_DKV, f), np.asarray(W_DQ, f)
    W_UK, W_UV, W_UQ = np.asarray(W_UK, f), np.asarray(W_UV, f), np.asarray(W_UQ, f)
    W_KR, W_QR, W_O = np.asarray(W_KR, f), np.asarray(W_QR, f), np.asarray(W_O, f)

    perm = np.concatenate([np.arange(0, DH, 2), np.arange(1, DH, 2)])
    w1 = np.ascontiguousarray(
        np.concatenate([W_DKV, W_DQ, W_KR[:, perm]], axis=1))
    xTs = [np.ascontiguousarray(x[b].T) for b in range(B)]

    inv = 1.0 / (10000.0 ** (np.arange(0, DH, 2, dtype=f) / DH))
    freqs = np.arange(L, dtype=f)[:, None] * inv[None, :]
    rope = np.concatenate([np.sin(freqs), np.cos(freqs)], axis=-1).astype(f)
    s_tab, c_tab = rope[:, 0::2], rope[:, 1::2]
    csT = np.ascontiguousarray(
        np.concatenate([c_tab.T, s_tab.T], axis=1))    # [64, 2L]

    maskbs = []
    for b in range(B):
        keep = (attention_mask[b] != 0).astype(f)      # 1=keep, 0=masked
        maskbs.append(np.ascontiguousarray(keep.reshape(KB, P).T))

    in_maps = []
    for c in range(8):
        b, hg = c // HG, c % HG
        cols = slice(hg * HDL, (hg + 1) * HDL)
        in_maps.append({
            "xT": xTs[b],
            "w1": w1,
            "wuk": np.ascontiguousarray(W_UK[:, cols]),
            "w3q": np.ascontiguousarray(
                np.concatenate([W_UQ[:, cols], W_QR[:, perm]], axis=1)),
            "wuv": np.ascontiguousarray(W_UV[:, cols]),
            "wo": np.ascontiguousarray(
                _BF16(W_O[hg * HDL:(hg + 1) * HDL, :])),
            "csT": _BF16(csT),
            "maskb": maskbs[b],
        })
    return in_maps


def kernel(x, attention_mask, W_DKV, W_DQ, W_UK, W_UV, W_UQ, W_KR, W_QR, W_O,
           **run_kwargs):
    in_maps = _host_prep(x, attention_mask, W_DKV, W_DQ, W_UK, W_UV, W_UQ,
                         W_KR, W_QR, W_O)
    nc = _get_nc()
    res = run_bass_kernel_spmd(nc, in_maps, core_ids=list(range(8)),
                               **run_kwargs)
    out = np.zeros((B, L, D), np.float32)
    for c in range(8):
        out[c // HG] += res.results[c]["out"]
    if run_kwargs:
        _CACHE["last_results"] = res
    return out


# revision 47
# speedup vs baseline: 1.0414x; 1.0157x over previous
"""MLA attention distributed over 8 TRN2 NeuronCores.

Sharding: tensor-parallel over heads (4 head-groups) x data-parallel over
batch (2). Each core computes, for its (batch, head-group):
  - the shared low-rank compressions c_kv/c_q and the rope key (replicated
    within a batch group),
  - K/V/Q up-projections for its 4 heads,
  - full attention for its 4 heads over all 2048 query positions,
  - a partial output projection (its heads' rows of W_O).
Host gather sums the 4 partial outputs per batch (row-parallel unshard).

Key structure (v3):
  - The rope score q_r.k_r is head-independent: computed once per
    (query-chunk, key-block-pair) and exponentiated once into T_r; per head
    the PE does a single 128-contraction matmul (q_c.k_c), the scalar engine
    exponentiates, and the DVE multiplies by T_r (exp(a+b)=exp(a)exp(b)).
  - The attention mask is folded into V: rows of the augmented [V|1] matrix
    are zeroed for masked keys, which removes the bias operand from every
    exp and makes the softmax denominator (from the ones column) exact.
  - Exps run on [128,1024] psum tiles spanning two banks (two score matmul
    groups per tile) to halve scalar-engine instruction overheads.
  - Query-chunk loop is outermost; the W_O projection of a finished chunk
    overlaps the next chunk's exp work.  Rope is computed per-chunk,
    interleaved into phase 1 / phase 3q so kr/qr are ready early.
"""

from contextlib import ExitStack

import numpy as np

import concourse.bacc as bacc
import concourse.mybir as mybir
import concourse.tile as tile
from concourse.bass_utils import run_bass_kernel_spmd
from concourse.masks import make_identity

B, L, D, H, DC, DH = 2, 2048, 2048, 16, 512, 128
HG = 4                 # head groups (tensor-parallel degree per batch)
HL = H // HG           # heads per core
HDL = HL * DH          # 512 head-dims per core
P = 128
N1 = 512               # matmul free-dim chunk
N2 = 1024              # fat (two-bank) psum tile width
F32 = mybir.dt.float32
BF16 = mybir.dt.bfloat16
F32R = mybir.dt.float32r
SCALE = 1.0 / float(np.sqrt(2 * DH))
M1 = 2 * DC + DH       # 1152: [W_DKV | W_DQ | W_KR] fused output rows
MT1 = M1 // P          # 9
KT1 = D // P           # 16
NCH = L // N1          # 4
KT3 = DC // P          # 4
KB = L // P            # 16 key blocks
KB2 = KB // 2          # 8 key-block pairs


def build_nc(debug=False):
    nc = bacc.Bacc(None, target_bir_lowering=False)

    xT = nc.dram_tensor("xT", [D, L], F32R, kind="ExternalInput")
    w1 = nc.dram_tensor("w1", [D, M1], F32R, kind="ExternalInput")
    wuk = nc.dram_tensor("wuk", [DC, HDL], F32R, kind="ExternalInput")
    w3q = nc.dram_tensor("w3q", [DC, HDL + DH], F32R, kind="ExternalInput")
    wuv = nc.dram_tensor("wuv", [DC, HDL], F32R, kind="ExternalInput")
    wo = nc.dram_tensor("wo", [HDL, D], BF16, kind="ExternalInput")
    # cos cols 0:L / sin cols L:2L on partitions 0:64 (transposed tables)
    cs_d = nc.dram_tensor("csT", [DH // 2, 2 * L], BF16, kind="ExternalInput")
    mask_d = nc.dram_tensor("maskb", [P, KB], F32, kind="ExternalInput")
    out_d = nc.dram_tensor("out", [L, D], F32, kind="ExternalOutput")

    with tile.TileContext(nc) as tc, ExitStack() as es:
        # ---------- constant + psum pools (live whole kernel) ----------
        p_const = es.enter_context(tc.tile_pool(name="const", bufs=1))
        p_ps_g = es.enter_context(tc.tile_pool(name="psg", bufs=2, space="PSUM"))
        p_ps_sc = es.enter_context(tc.tile_pool(name="pssc", bufs=2, space="PSUM"))
        p_ps_av = es.enter_context(tc.tile_pool(name="psav", bufs=2, space="PSUM"))

        km_t = p_const.tile([P, KB], F32, name="km_t")   # 0/1 keep-mask
        nc.sync.dma_start(km_t[:], mask_d[:])
        ident = p_const.tile([P, P], BF16, name="ident")
        make_identity(nc, ident[:])
        warm = p_const.tile([P, 1], F32, name="warm")
        nc.scalar.activation(warm[:], km_t[:, 0:1],
                             mybir.ActivationFunctionType.Exp)

        # ---------- long-lived rope outputs (right-side stack) ----------
        es_krqr = ExitStack()
        p_krqr = es_krqr.enter_context(tc.tile_pool(name="krqrp", bufs=1, side="right"))
        kr_t = [p_krqr.tile([P, N1], BF16, name=f"krT{c}", tag=f"krT{c}")
                for c in range(NCH)]
        qr_t = [p_krqr.tile([P, N1], BF16, name=f"qrT{c}", tag=f"qrT{c}")
                for c in range(NCH)]

        # ---------- phase-1 residents ----------
        es_ckv = ExitStack()
        p_ckv = es_ckv.enter_context(tc.tile_pool(name="ckvp", bufs=1))
        es_cq = ExitStack()
        p_cq = es_cq.enter_context(tc.tile_pool(name="cqp", bufs=1))
        ckv_t = [p_ckv.tile([P, L], F32R, name=f"ckv{i}", tag=f"ckv{i}")
                 for i in range(KT3)]
        cq_t = [p_cq.tile([P, L], F32R, name=f"cq{i}", tag=f"cq{i}")
                for i in range(KT3)]

        # rope tables + pre-rope tiles: dead after the rope work, popped
        # before 3v.
        es_tabxr = ExitStack()
        p_tab = es_tabxr.enter_context(tc.tile_pool(name="tabp", bufs=1))
        p_rope = es_tabxr.enter_context(tc.tile_pool(name="ropep", bufs=1))
        p_xr = es_tabxr.enter_context(tc.tile_pool(name="xrp", bufs=1))
        cs_t = p_tab.tile([DH // 2, 2 * L], BF16, name="cs_t")  # [cos|sin]
        nc.sync.dma_start(cs_t[:], cs_d[:])
        xrk_t = p_xr.tile([P, L], BF16, name="xrkT")

        # one rope chunk: dst[0:64] = xe*c - xo*s ; dst[64:128] = xe*s + xo*c
        # (xo staged down to partitions 0:64, second half staged back up)
        def rope_chunk(src_t, dst_t, ch, pfx, eng):
            cs = slice(ch * N1, (ch + 1) * N1)
            dst = dst_t[ch]
            xe = src_t[0:64, cs]
            xo = p_rope.tile([64, N1], BF16, tag="rxo", name=f"{pfx}xo{ch}")
            nc.gpsimd.dma_start(xo[:], src_t[64:128, cs])
            cc = cs_t[:, ch * N1:(ch + 1) * N1]
            ss = cs_t[:, L + ch * N1:L + (ch + 1) * N1]
            t1 = p_rope.tile([64, N1], F32, tag="rt1", name=f"{pfx}t1{ch}")
            t2 = p_rope.tile([64, N1], F32, tag="rt2", name=f"{pfx}t2{ch}")
            eng.tensor_tensor(t1[:], xe, cc, mybir.AluOpType.mult)
            eng.tensor_tensor(t2[:], xo[:], ss, mybir.AluOpType.mult)
            eng.tensor_tensor(dst[0:64, :], t1[:], t2[:],
                              mybir.AluOpType.subtract)
            t3 = p_rope.tile([64, N1], F32, tag="rt1", name=f"{pfx}t3{ch}")
            t4 = p_rope.tile([64, N1], F32, tag="rt2", name=f"{pfx}t4{ch}")
            eng.tensor_tensor(t3[:], xe, ss, mybir.AluOpType.mult)
            eng.tensor_tensor(t4[:], xo[:], cc, mybir.AluOpType.mult)
            h2 = p_rope.tile([64, N1], BF16, tag="rh2", name=f"{pfx}h2{ch}")
            eng.tensor_tensor(h2[:], t3[:], t4[:], mybir.AluOpType.add)
            nc.gpsimd.dma_start(dst[64:128, :], h2[:])

        # ---------- phase 1: c_kvT | c_qT | xrkT = [Wdkv|Wdq|Wkr].T @ x.T ----
        es_w1 = ExitStack()
        p_w1 = es_w1.enter_context(tc.tile_pool(name="w1p", bufs=1))
        es_xn = ExitStack()
        p_xn = es_xn.enter_context(tc.tile_pool(name="xnp", bufs=20))

        # x tiles alternate sync/vector queues; w1 first-column slices land
        # first (on scalar), then the rest in two stages, so the PE mt-loop
        # is never starved.
        w1_t = []
        xts0 = []
        for kt in range(KT1):
            t = p_xn.tile([P, N1], F32R, tag="xn", name=f"xn_0_{kt}")
            nc.sync.dma_start(t[:], xT[kt * P:(kt + 1) * P, 0:N1])
            xts0.append(t)
            t = p_w1.tile([P, M1], F32R, name=f"w1_{kt}", tag=f"w1_{kt}")
            nc.scalar.dma_start(t[:, 0:P], w1[kt * P:(kt + 1) * P, 0:P])
            w1_t.append(t)
        for lo, hi in ((P, 5 * P), (5 * P, M1)):
            for kt in range(KT1):
                nc.scalar.dma_start(
                    w1_t[kt][:, lo:hi], w1[kt * P:(kt + 1) * P, lo:hi])

        dest1 = ckv_t + cq_t + [xrk_t]
        for nci in range(NCH):
            if nci == 0:
                xts = xts0
            else:
                xts = []
                for kt in range(KT1):
                    t = p_xn.tile([P, N1], F32R, tag="xn", name=f"xn_{nci}_{kt}")
                    nc.sync.dma_start(
                        t[:], xT[kt * P:(kt + 1) * P, nci * N1:(nci + 1) * N1])
                    xts.append(t)
            mt_order = ([4, 5, 6, 7, 0, 1, 2, 3, 8] if nci == NCH - 1
                        else list(range(MT1)))
            for mt in mt_order:
                ps = p_ps_g.tile([P, N1], F32, tag="g", name=f"ps1_{nci}_{mt}")
                for kt in range(KT1):
                    nc.tensor.matmul(ps[:], w1_t[kt][:, mt * P:(mt + 1) * P],
                                     xts[kt][:],
                                     start=(kt == 0), stop=(kt == KT1 - 1))
                # split evictions between DVE and ACT (both idle-ish here)
                if mt % 2 == 0:
                    nc.vector.tensor_copy(dest1[mt][:, nci * N1:(nci + 1) * N1],
                                          ps[:])
                else:
                    nc.scalar.activation(dest1[mt][:, nci * N1:(nci + 1) * N1],
                                         ps[:],
                                         mybir.ActivationFunctionType.Copy)
            # rope-k for this chunk (kr ready long before attention)
            rope_chunk(xrk_t, kr_t, nci, "k", nc.vector)
        es_xn.close()
        es_w1.close()

        es_qc = ExitStack()
        p_qc = es_qc.enter_context(tc.tile_pool(name="qcp", bufs=1, side="right"))
        qc_t = [p_qc.tile([P, L], F32R, tag=f"qc{i}", name=f"qc{i}")
                for i in range(HL)]

        # ---------- phase 3q: q_cT | xrqT = [Wuq_hg|Wqr].T @ c_qT ----------
        es_wuk = ExitStack()
        p_wuk = es_wuk.enter_context(tc.tile_pool(name="wukp", bufs=1))
        wuk_t = []
        for kt in range(KT3):
            t = p_wuk.tile([P, HDL], F32R, tag=f"wuk{kt}", name=f"wuk{kt}")
            nc.scalar.dma_start(t[:], wuk[kt * P:(kt + 1) * P, :])
            wuk_t.append(t)
        es_w3q = ExitStack()
        p_w3q = es_w3q.enter_context(tc.tile_pool(name="w3qp", bufs=1))
        w3q_t = []
        for kt in range(KT3):
            t = p_w3q.tile([P, HDL + DH], F32R, tag=f"w3q{kt}", name=f"w3q{kt}")
            nc.scalar.dma_start(t[:, 0:P], w3q[kt * P:(kt + 1) * P, 0:P])
            w3q_t.append(t)
        for kt in range(KT3):
            nc.scalar.dma_start(w3q_t[kt][:, P:HDL + DH],
                                w3q[kt * P:(kt + 1) * P, P:HDL + DH])
        xrq_t = p_xr.tile([P, L], BF16, name="xrqT")
        dest3 = qc_t + [xrq_t]
        for nci in range(NCH):
            for mt in range(HL + 1):
                pool, tg = ((p_ps_g, "g") if (nci * (HL + 1) + mt) % 2 == 0
                            else (p_ps_sc, "sc"))
                ps = pool.tile([P, N1], F32, tag=tg, name=f"ps3_{nci}_{mt}")
                for kt in range(KT3):
                    nc.tensor.matmul(ps[:], w3q_t[kt][:, mt * P:(mt + 1) * P],
                                     cq_t[kt][:, nci * N1:(nci + 1) * N1],
                                     start=(kt == 0), stop=(kt == KT3 - 1))
                if mt % 2 == 0:
                    nc.vector.tensor_copy(dest3[mt][:, nci * N1:(nci + 1) * N1],
                                          ps[:])
                else:
                    nc.scalar.activation(dest3[mt][:, nci * N1:(nci + 1) * N1],
                                         ps[:],
                                         mybir.ActivationFunctionType.Copy)
            # rope-q for this chunk (qr ready before attention needs it)
            rope_chunk(xrq_t, qr_t, nci, "q", nc.gpsimd)
        es_w3q.close()

        # ---------- phase 3k: k_cT = Wuk_hg.T @ c_kvT ----------
        es_kc = ExitStack()
        p_kc = es_kc.enter_context(tc.tile_pool(name="kcp", bufs=1, side="right"))
        kc_t = [p_kc.tile([P, L], F32R, tag=f"kc{i}", name=f"kc{i}")
                for i in range(HL)]
        for nci in range(NCH):
            for mt in range(HL):
                pool, tg = ((p_ps_g, "g") if (nci * HL + mt) % 2 == 0
                            else (p_ps_sc, "sc"))
                ps = pool.tile([P, N1], F32, tag=tg, name=f"ps3k_{nci}_{mt}")
                for kt in range(KT3):
                    nc.tensor.matmul(ps[:], wuk_t[kt][:, mt * P:(mt + 1) * P],
                                     ckv_t[kt][:, nci * N1:(nci + 1) * N1],
                                     start=(kt == 0), stop=(kt == KT3 - 1))
                if mt % 2 == 0:
                    nc.vector.tensor_copy(kc_t[mt][:, nci * N1:(nci + 1) * N1],
                                          ps[:])
                else:
                    nc.scalar.activation(kc_t[mt][:, nci * N1:(nci + 1) * N1],
                                         ps[:],
                                         mybir.ActivationFunctionType.Copy)
        es_wuk.close()
        es_tabxr.close()   # rope tables + xr + rope temps dead from here
        es_cq.close()

        # ---------- phase 3v: v = c_kv @ Wuv_hg (natural), bf16 + ones col ---
        # masked keys' rows (incl. the ones column) are zeroed -> softmax
        # ignores them and the denominator stays exact.
        es_v = ExitStack()
        p_v = es_v.enter_context(tc.tile_pool(name="vp", bufs=1, side="right"))
        es_wuv = ExitStack()
        p_wuv = es_wuv.enter_context(tc.tile_pool(name="wuvp", bufs=1))
        wuv_t = []
        for kt in range(KT3):
            t = p_wuv.tile([P, HDL], F32R, tag=f"wuv{kt}", name=f"wuv{kt}")
            nc.scalar.dma_start(t[:], wuv[kt * P:(kt + 1) * P, :])
            wuv_t.append(t)
        vaug_t = [p_v.tile([P, HL * (DH + 1)], BF16, tag=f"v{i}", name=f"v{i}")
                  for i in range(KB)]
        # first query-chunk's shared rope score, emitted here so its T_r
        # exps on the scalar engine hide under 3v's PE matmuls
        trs0 = []
        for kb2 in range(KB2):
            ps = p_ps_sc.tile([P, N2], F32, tag="sc", name=f"sr_0_{kb2}")
            for j in range(2):
                kb = 2 * kb2 + j
                nc.tensor.matmul(
                    ps[:, j * N1:(j + 1) * N1],
                    kr_t[kb // 4][:, (kb % 4) * P:(kb % 4 + 1) * P],
                    qr_t[0][:], start=True, stop=True)
            tr = p_v.tile([P, N2], BF16, tag=f"tr0_{kb2}", name=f"tr_0_{kb2}")
            nc.scalar.activation(tr[:], ps[:],
                                 mybir.ActivationFunctionType.Exp,
                                 scale=SCALE)
            trs0.append(tr)
        for mt in range(KB):
            pool, tg = (p_ps_g, "g") if mt % 2 == 0 else (p_ps_sc, "sc")
            ps = pool.tile([P, N1], F32, tag=tg, name=f"psv_{mt}")
            for kt in range(KT3):
                nc.tensor.matmul(ps[:], ckv_t[kt][:, mt * P:(mt + 1) * P],
                                 wuv_t[kt][:],
                                 start=(kt == 0), stop=(kt == KT3 - 1))
            va = vaug_t[mt].rearrange("p (h c) -> p h c", c=DH + 1)
            nc.vector.tensor_copy(va[:, :, 0:DH],
                                  ps.rearrange("p (h c) -> p h c", c=DH))
            nc.vector.memset(va[:, :, DH:DH + 1], 1.0)
            nc.vector.tensor_scalar_mul(vaug_t[mt][:], vaug_t[mt][:],
                                        km_t[:, mt:mt + 1])
        es_wuv.close()
        es_ckv.close()

        # ---------- W_O cache (bf16; queue is idle here) ----------
        p_wo = es.enter_context(tc.tile_pool(name="wop", bufs=1))
        wo_t = {}
        for nci in range(NCH):
            for kt in range(HL):
                t = p_wo.tile([P, N1], BF16, tag=f"wo{nci}_{kt}",
                              name=f"wo_{nci}_{kt}")
                nc.sync.dma_start(t[:], wo[kt * P:(kt + 1) * P,
                                            nci * N1:(nci + 1) * N1])
                wo_t[(nci, kt)] = t

        # ---------- phase 4+5: attention, query-chunk outer ----------
        p_ctx = es.enter_context(tc.tile_pool(name="ctxp", bufs=1))
        es_exp = ExitStack()
        p_e = es_exp.enter_context(tc.tile_pool(name="expp", bufs=12))
        es_tr = ExitStack()
        p_tr = es_tr.enter_context(tc.tile_pool(name="trp", bufs=9))
        es_sm = ExitStack()
        p_sm = es_sm.enter_context(tc.tile_pool(name="smallp", bufs=12))
        es_st = ExitStack()
        p_st = es_st.enter_context(tc.tile_pool(name="stagep", bufs=2))

        ctx_t = [p_ctx.tile([P, L], BF16, tag=f"ctxT{h}", name=f"ctxT{h}")
                 for h in range(HL)]

        # Software-pipelined emission: head h's AV matmuls are interleaved
        # between head h+1's score tiles so the PE has work while the scalar
        # engine exponentiates; the last head's AV interleaves with the NEXT
        # chunk's rope-score matmuls, and phase 5 of the finished chunk runs
        # while the scalar engine starts on the next chunk's T_r/exps.
        def emit_sr_tile(qch, kb2):
            ps = p_ps_sc.tile([P, N2], F32, tag="sc", name=f"sr_{qch}_{kb2}")
            for j in range(2):
                kb = 2 * kb2 + j
                nc.tensor.matmul(
                    ps[:, j * N1:(j + 1) * N1],
                    kr_t[kb // 4][:, (kb % 4) * P:(kb % 4 + 1) * P],
                    qr_t[qch][:], start=True, stop=True)
            tr = p_tr.tile([P, N2], BF16, tag="trT", name=f"tr_{qch}_{kb2}")
            nc.scalar.activation(tr[:], ps[:],
                                 mybir.ActivationFunctionType.Exp,
                                 scale=SCALE)
            return tr

        def emit_score_tile(qch, h, kb2, trs):
            qsl = slice(qch * N1, (qch + 1) * N1)
            ps = p_ps_sc.tile([P, N2], F32, tag="sc",
                              name=f"sc_{h}_{qch}_{kb2}")
            for j in range(2):
                kb = 2 * kb2 + j
                nc.tensor.matmul(ps[:, j * N1:(j + 1) * N1],
                                 kc_t[h][:, kb * P:(kb + 1) * P],
                                 qc_t[h][:, qsl], start=True, stop=True)
            et = p_e.tile([P, N2], BF16, tag="expT",
                          name=f"et_{h}_{qch}_{kb2}")
            nc.scalar.activation(et[:], ps[:],
                                 mybir.ActivationFunctionType.Exp,
                                 scale=SCALE)
            nc.vector.tensor_tensor(et[:], et[:], trs[kb2][:],
                                    mybir.AluOpType.mult)
            return et

        def emit_av_part(qch, h, i, exps, pcs):
            # slot i of 8: qc = i//2, key blocks (i%2)*8 .. +8
            qc = i // 2
            q0 = qch * 4 + qc
            if i % 2 == 0:
                pcs[qc] = p_ps_av.tile([P, DH + 1], F32, tag="av",
                                       name=f"av_{h}_{q0}")
            pc = pcs[qc]
            for kb in range((i % 2) * 8, (i % 2) * 8 + 8):
                nc.tensor.matmul(
                    pc[:],
                    exps[kb // 2][:, (kb % 2) * N1 + qc * P:
                                  (kb % 2) * N1 + (qc + 1) * P],
                    vaug_t[kb][:, h * (DH + 1):(h + 1) * (DH + 1)],
                    start=(kb == 0), stop=(kb == KB - 1))
            if i % 2 == 1:
                rc = p_sm.tile([P, 1], F32, tag="recip", name=f"rc_{h}_{q0}")
                nc.vector.reciprocal(rc[:], pc[:, DH:DH + 1])
                cn = p_sm.tile([P, DH], BF16, tag="cn", name=f"cn_{h}_{q0}")
                nc.vector.tensor_scalar_mul(cn[:], pc[:, 0:DH], rc[:])
                pt = p_ps_g.tile([P, P], BF16, tag="g", name=f"tp_{h}_{q0}")
                nc.tensor.transpose(pt[:], cn[:], ident[:])
                nc.vector.tensor_copy(ctx_t[h][:, q0 * P:(q0 + 1) * P],
                                      pt[:])

        ph5_stg = {}

        def emit_ph5_group(qch, g):
            # one of 16 psum groups for chunk qch: g = qc*4 + nci
            qc, nci = g // NCH, g % NCH
            q0 = qch * 4 + qc
            if nci == 0:
                ph5_stg[q0] = p_st.tile([P, D], F32, tag="stage",
                                        name=f"st_{q0}")
            stg = ph5_stg[q0]
            ps = p_ps_g.tile([P, N1], F32, tag="g", name=f"ps5_{q0}_{nci}")
            for kt in range(HL):
                nc.tensor.matmul(ps[:], ctx_t[kt][:, q0 * P:(q0 + 1) * P],
                                 wo_t[(nci, kt)][:],
                                 start=(kt == 0), stop=(kt == HL - 1))
            nc.vector.tensor_copy(stg[:, nci * N1:(nci + 1) * N1], ps[:])
            if nci == NCH - 1:
                nc.scalar.dma_start(out_d[q0 * P:(q0 + 1) * P, :], stg[:])

        trs = trs0
        exps_pre = []
        for qch in range(NCH):
            exps = exps_pre + [emit_score_tile(qch, 0, kb2, trs)
                               for kb2 in range(len(exps_pre), KB2)]
            for h in range(HL - 1):
                nexps, pcs = [], {}
                for i in range(KB2):
                    nexps.append(emit_score_tile(qch, h + 1, i, trs))
                    emit_av_part(qch, h, i, exps, pcs)
                exps = nexps
            # last head's AV interleaves with the next chunk's rope scores
            ntrs, pcs = [], {}
            for i in range(KB2):
                if qch + 1 < NCH:
                    ntrs.append(emit_sr_tile(qch + 1, i))
                emit_av_part(qch, HL - 1, i, exps, pcs)
            trs = ntrs
            # pre-emit next chunk's first head-0 score tiles: their exps
            # fill the scalar engine's idle window during the ph5 block
            exps_pre = ([emit_score_tile(qch + 1, 0, i, trs) for i in range(4)]
                        if qch + 1 < NCH else [])
            for g in range(4 * NCH):
                emit_ph5_group(qch, g)

        es_st.close()
        es_sm.close()
        es_tr.close()
        es_exp.close()
        es_v.close()
        es_kc.close()
        es_qc.close()
        es_krqr.close()

    nc.compile()
    return nc


_CACHE = {}


def _get_nc():
    if "nc" not in _CACHE:
        _CACHE["nc"] = build_nc()
    return _CACHE["nc"]


def _BF16(a):
    import ml_dtypes
    return np.asarray(a, dtype=ml_dtypes.bfloat16)


def _host_prep(x, attention_mask, W_DKV, W_DQ, W_UK, W_UV, W_UQ, W_KR, W_QR,
               W_O):
    f = np.float32
    x = np.asarray(x, f)
    attention_mask = np.asarray(attention_mask)
    W_DKV, W_DQ = np.asarray(W_DKV, f), np.asarray(W_DQ, f)
    W_UK, W_UV, W_UQ = np.asarray(W_UK, f), np.asarray(W_UV, f), np.asarray(W_UQ, f)
    W_KR, W_QR, W_O = np.asarray(W_KR, f), np.asarray(W_QR, f), np.asarray(W_O, f)

    perm = np.concatenate([np.arange(0, DH, 2), np.arange(1, DH, 2)])
    w1 = np.ascontiguousarray(
        np.concatenate([W_DKV, W_DQ, W_KR[:, perm]], axis=1))
    xTs = [np.ascontiguousarray(x[b].T) for b in range(B)]

    inv = 1.0 / (10000.0 ** (np.arange(0, DH, 2, dtype=f) / DH))
    freqs = np.arange(L, dtype=f)[:, None] * inv[None, :]
    rope = np.concatenate([np.sin(freqs), np.cos(freqs)], axis=-1).astype(f)
    s_tab, c_tab = rope[:, 0::2], rope[:, 1::2]
    csT = np.ascontiguousarray(
        np.concatenate([c_tab.T, s_tab.T], axis=1))    # [64, 2L]

    maskbs = []
    for b in range(B):
        keep = (attention_mask[b] != 0).astype(f)      # 1=keep, 0=masked
        maskbs.append(np.ascontiguousarray(keep.reshape(KB, P).T))

    in_maps = []
    for c in range(8):
        b, hg = c // HG, c % HG
        cols = slice(hg * HDL, (hg + 1) * HDL)
        in_maps.append({
            "xT": xTs[b],
            "w1": w1,
            "wuk": np.ascontiguousarray(W_UK[:, cols]),
            "w3q": np.ascontiguousarray(
                np.concatenate([W_UQ[:, cols], W_QR[:, perm]], axis=1)),
            "wuv": np.ascontiguousarray(W_UV[:, cols]),
            "wo": np.ascontiguousarray(
                _BF16(W_O[hg * HDL:(hg + 1) * HDL, :])),
            "csT": _BF16(csT),
            "maskb": maskbs[b],
        })
    return in_maps


def kernel(x, attention_mask, W_DKV, W_DQ, W_UK, W_UV, W_UQ, W_KR, W_QR, W_O,
           **run_kwargs):
    in_maps = _host_prep(x, attention_mask, W_DKV, W_DQ, W_UK, W_UV, W_UQ,
                         W_KR, W_QR, W_O)
    nc = _get_nc()
    res = run_bass_kernel_spmd(nc, in_maps, core_ids=list(range(8)),
                               **run_kwargs)
    out = np.zeros((B, L, D), np.float32)
    for c in range(8):
        out[c // HG] += res.results[c]["out"]
    if run_kwargs:
        _CACHE["last_results"] = res
    return out
